# revision 63
# baseline (speedup 1.0000x reference)
"""BiLSTM-CRF sequence-tagging loss on 8 Trainium2 NeuronCores.

Sharding: 8 cores = 4 batch-groups x 2 LSTM directions.
  core 2g+d handles sequences [8g, 8g+8) ; d=0 forward, d=1 backward.
Backward cores receive time-reversed inputs (ids/pos/labels), so one SPMD
program runs on all cores; their CRF uses transposed transitions with
start/end swapped (same loss by path reversal), and their layer-2/emission
weights are column-permuted so the local [own_h, partner_h] concat order is
uniform.  The h-streams are exchanged pairwise via AllGather through DRAM;
the partner slot is fetched with an indirect-DMA row gather whose indices
are per-core input data (keeps the program core-uniform).
"""

import os
import sys

import numpy as np

for _p in ("/opt/trn_rl_repo", "/root/.axon_site/_ro/trn_rl_repo"):
    if os.path.isdir(_p) and _p not in sys.path:
        sys.path.insert(0, _p)

import ml_dtypes  # noqa: E402

import concourse.bass as bass  # noqa: E402
import concourse.bacc as bacc  # noqa: E402
import concourse.tile as tile  # noqa: E402
from concourse import mybir  # noqa: E402
from concourse.bass import IndirectOffsetOnAxis  # noqa: E402
from concourse.bass_utils import run_bass_kernel_spmd  # noqa: E402
from concourse.masks import make_identity  # noqa: E402

F32 = mybir.dt.float32
BF16 = mybir.dt.bfloat16
F8E4 = mybir.dt.float8e4
I32 = mybir.dt.int32
AF = mybir.ActivationFunctionType
ALU = mybir.AluOpType

# problem shapes (hardcoded per contract)
B, T, V, D, C, HD = 32, 256, 30522, 768, 14, 384
L = 2
NCORES = 8
GB = 8            # sequences per core group
NT = GB * T       # tokens per core = 2048
NTILE = NT // 128  # 16
MCH = 12          # gate chunks of 128 (4*HD/128)
KCH = 3           # hidden chunks (HD/128)
DCH = 6           # input-dim chunks (D/128)
LN_EPS = 1e-12
PAIRS = [[0, 1], [2, 3], [4, 5], [6, 7]]
ESCALE = 16.0     # folded per-step scaling of exp(trans); no renorms needed
NBLK = 4          # CRF scan blocks (1 alpha chain + NBLK-1 matrix chains)
BT = T // NBLK    # 64 steps per block
CPW = 34 + 2 * 14  # cpack width
NCHUNK = 4        # exchange chunks per layer
QT = T // NCHUNK  # 64 timesteps per chunk
QN = GB * QT      # 512 columns per chunk
SCH = GB // 2     # sequences per recurrence chain
NBG = NT // 512   # 512-token (64-step) blocks

DEBUG_OUTS = False


def _bf(x):
    return np.ascontiguousarray(np.asarray(x, dtype=np.float32)).astype(ml_dtypes.bfloat16)


def _f32(x):
    return np.ascontiguousarray(np.asarray(x, dtype=np.float32))


def _f8(x):
    return np.ascontiguousarray(np.asarray(x, dtype=np.float32)).astype(
        ml_dtypes.float8_e4m3
    )


def _perm_gates(w):
    """torch gate order i,f,g,o -> device order i,f,o,g with the g block
    scaled by 2 (tanh(x) = 2*sigmoid(2x)-1)."""
    H = HD
    return np.concatenate([w[0:H], w[H:2 * H], w[3 * H:4 * H], 2.0 * w[2 * H:3 * H]], axis=0)


# ---------------------------------------------------------------------------
# device program
# ---------------------------------------------------------------------------

def build_program():
    nc = bacc.Bacc("TRN2", target_bir_lowering=False, debug=False, num_devices=NCORES)

    def din(name, shape, dt):
        return nc.dram_tensor(name, shape, dt, kind="ExternalInput").ap()

    ins = dict(
        ids32=din("ids32", [NT, 1], I32),
        labf=din("labf", [1, NT], F32),
        word_emb=din("word_emb", [V, D], BF16),
        posty=din("posty", [NT, D], BF16),
        wih0T=din("wih0T", [D, 4 * HD], F8E4),
        wih1T=din("wih1T", [D, 4 * HD], F8E4),
        whh0T=din("whh0T", [HD, 4 * HD], BF16),
        whh1T=din("whh1T", [HD, 4 * HD], BF16),
        b01=din("b01", [128, 2 * MCH], F32),
        fcT=din("fcT", [D, C], F8E4),
        cpack=din("cpack", [C, CPW], F32),
        gidx=din("gidx", [128, KCH], I32),
    )

    loss_out = nc.dram_tensor("loss", [1, 1], F32, kind="ExternalOutput").ap()
    dbg = {}
    if DEBUG_OUTS:
        dbg["dbg_xt"] = nc.dram_tensor("dbg_xt", [128, DCH, NT], BF16, kind="ExternalOutput").ap()
        dbg["dbg_g"] = nc.dram_tensor("dbg_g", [128, MCH, NT], BF16, kind="ExternalOutput").ap()
        dbg["dbg_h1"] = nc.dram_tensor("dbg_h1", [128, KCH, NT], BF16, kind="ExternalOutput").ap()
        dbg["dbg_h2"] = nc.dram_tensor("dbg_h2", [128, KCH, NT], BF16, kind="ExternalOutput").ap()
        dbg["dbg_em"] = nc.dram_tensor("dbg_em", [C, NT], F32, kind="ExternalOutput").ap()
        dbg["dbg_sc"] = nc.dram_tensor("dbg_sc", [1, 2], F32, kind="ExternalOutput").ap()

    # internal DRAM for pairwise exchange (fp8, 4 time-chunks per layer so
    # collectives overlap the recurrence)
    ctrb = [
        [nc.dram_tensor(f"ctrb{l}_{j}", [KCH, 128, QN], F8E4) for j in range(NCHUNK)]
        for l in range(L)
    ]
    hall = [
        [nc.dram_tensor(f"hall{l}_{j}", [2, KCH, 128, QN], F8E4) for j in range(NCHUNK)]
        for l in range(L)
    ]

    with tile.TileContext(nc) as tc:
        _build_body(tc, ins, loss_out, dbg, ctrb, hall)

    nc.compile()
    return nc


def _build_body(tc, ins, loss_out, dbg, ctrb, hall):
    nc = tc.nc
    from contextlib import ExitStack

    est = ExitStack()
    pers = est.enter_context(tc.tile_pool(name="pers", bufs=1))

    # ---- persistent SBUF state (small constants only) ----
    def load_wih(l, pool):
        wt = pool.tile([128, DCH, 4 * HD], F8E4, name=f"wih{l}")
        src = ins["wih0T"] if l == 0 else ins["wih1T"]
        nc.sync.dma_start(out=wt[:], in_=src.rearrange("(k p) m -> p k m", p=128))
        return wt

    def load_whh(l, pool):
        ht = pool.tile([128, KCH, 4 * HD], BF16, name=f"whh{l}")
        src = ins["whh0T"] if l == 0 else ins["whh1T"]
        nc.sync.dma_start(out=ht[:], in_=src.rearrange("(k p) m -> p k m", p=128))
        return ht

    # scratch + absorbers: this toolchain allows only ONE sem wait per
    # instruction, so every junction of two producers gets a tiny absorber op
    # that folds one producer into the consuming engine's clock first.
    scr_dve = pers.tile([1, 4], F32, name="scr_dve")
    scr_gp = pers.tile([1, 4], I32, name="scr_gp")
    pabs = est.enter_context(tc.tile_pool(name="pabs", bufs=1, space="PSUM"))
    pscr = pabs.tile([1, 8], F32, name="pscr")

    def dve_touch(ap):
        nc.vector.tensor_copy(out=scr_dve[:, 0:1], in_=ap)

    def pe_touch_f32(ap_col):
        nc.tensor.matmul(out=pscr[:1, :1], lhsT=ap_col, rhs=ap_col, start=True, stop=True)

    b_sb = pers.tile([128, 2 * MCH], F32, name="b_sb")
    nc.sync.dma_start(out=b_sb[:], in_=ins["b01"])
    dve_touch(b_sb[0:1, 0:1])

    fcT_sb = pers.tile([128, DCH, C], F8E4, name="fcT")
    nc.sync.dma_start(out=fcT_sb[:], in_=ins["fcT"].rearrange("(k p) m -> p k m", p=128))

    cpack_sb = pers.tile([C, CPW], F32, name="cpack_sb")
    nc.sync.dma_start(out=cpack_sb[:], in_=ins["cpack"])
    dve_touch(cpack_sb[0:1, 0:1])
    E_sb = cpack_sb[:, 0:C]  # exp(trans)/ESCALE
    transT_sb = cpack_sb[:, C : 2 * C]
    expst_sb = cpack_sb[:, 28:29]
    expen_sb = cpack_sb[:, 29:30]
    stv_sb = cpack_sb[:, 30:31]
    env_sb = cpack_sb[:, 31:32]
    iota_sb = cpack_sb[:, 32:33]
    fcb_sb = cpack_sb[:, 33:34]
    expTTp_f32 = cpack_sb[:, 34 : 34 + C]  # exp(trans.T)/ESCALE
    eye_sb = cpack_sb[:, 34 + C : 34 + 2 * C]  # identity

    gidx_sb = pers.tile([128, KCH], I32, name="gidx_sb")
    nc.sync.dma_start(out=gidx_sb[:], in_=ins["gidx"])
    nc.gpsimd.tensor_copy(out=scr_gp[:, 0:1], in_=gidx_sb[0:1, 0:1])

    ids_sb = pers.tile([128, NTILE], I32, name="ids_sb")
    nc.sync.dma_start(out=ids_sb[:], in_=ins["ids32"].rearrange("(k p) o -> p (k o)", p=128))

    ident = pers.tile([128, 128], F32, name="ident")
    make_identity(nc, ident[:])
    ident_bf = pers.tile([128, 128], BF16, name="ident_bf")
    nc.vector.tensor_copy(out=ident_bf[:], in_=ident[:])
    pe_touch_f32(ident[:, 0:1])
    eps_sb = pers.tile([128, 1], F32, name="eps_sb")
    nc.vector.memset(eps_sb[:], LN_EPS)
    ones1C = pers.tile([1, C], F32, name="ones1C")
    nc.vector.memset(ones1C[:], 1.0)
    onesC1 = pers.tile([C, 1], F32, name="onesC1")
    nc.vector.memset(onesC1[:], 1.0)

    # ---- helpers ----
    def make_embed_ktile(s1, s1ps, xT_t):
        """Returns emit(k): embeds token k-tile k (128 t-major tokens) into
        xT_t[k//4]. Called lazily so ktiles can be woven into the layer-0
        recurrence's engine slack."""

        def emit(k):
            posty_sb = s1.tile([128, D], BF16, tag="posty")
            nc.sync.dma_start(
                out=posty_sb[:], in_=ins["posty"][128 * k : 128 * (k + 1), :]
            )
            emb = s1.tile([128, D], BF16, tag="emb")
            nc.gpsimd.indirect_dma_start(
                out=emb[:],
                out_offset=None,
                in_=ins["word_emb"],
                in_offset=IndirectOffsetOnAxis(ap=ids_sb[:, k : k + 1], axis=0),
            )
            emb2 = s1.tile([128, D], BF16, tag="emb2")
            nc.vector.tensor_add(out=emb2[:], in0=emb[:], in1=posty_sb[:])
            stats = s1.tile([128, 3, 6], F32, tag="stats")
            embv = emb2[:].rearrange("p (s q) -> p s q", s=3)
            for sg in range(3):
                nc.vector.bn_stats(out=stats[:, sg, :], in_=embv[:, sg, :])
            mv = s1.tile([128, 2], F32, tag="mv")
            nc.vector.bn_aggr(out=mv[:], in_=stats[:])
            std = s1.tile([128, 1], F32, tag="std")
            nc.scalar.activation(out=std[:], in_=mv[:, 1:2], func=AF.Sqrt, bias=eps_sb[:])
            rstd = s1.tile([128, 1], F32, tag="rstd")
            nc.vector.reciprocal(out=rstd[:], in_=std[:])
            xln = s1.tile([128, D], BF16, tag="xln")
            nc.vector.tensor_scalar(
                out=xln[:],
                in0=emb2[:],
                scalar1=mv[:, 0:1],
                scalar2=rstd[:],
                op0=ALU.subtract,
                op1=ALU.mult,
            )
            for j in range(DCH):
                tp = s1ps.tile([128, 128], BF16, tag="tp")
                nc.tensor.transpose(
                    out=tp[:], in_=xln[:, 128 * j : 128 * (j + 1)], identity=ident_bf[:]
                )
                # spread PSUM->SBUF copies across engines; DVE is the
                # embed bottleneck
                dst = xT_t[k // 4][:, j, 128 * (k % 4) : 128 * (k % 4 + 1)]
                if j % 2 == 0:
                    nc.vector.tensor_copy(out=dst, in_=tp[:])
                else:
                    nc.scalar.copy(out=dst, in_=tp[:])

        return emit

    DR = mybir.MatmulPerfMode.DoubleRow

    def g0_mgroup(G_t, xT_t, wih, gps, nb, m):
        """One gate-chunk m of G0 block nb: 3 fp8 DoubleRow matmuls + bias."""
        ms = slice(128 * m, 128 * (m + 1))
        ps = gps.tile([128, 512], F32, tag="gps")
        for kp in range(DCH // 2):
            nc.tensor.matmul(
                out=ps[:],
                lhsT=wih[:, 2 * kp : 2 * kp + 2, ms],
                rhs=xT_t[nb][:, 2 * kp : 2 * kp + 2, :],
                start=(kp == 0),
                stop=(kp == DCH // 2 - 1),
                perf_mode=DR,
            )
        nc.vector.tensor_scalar_add(
            out=G_t[nb][:, m, :], in0=ps[:], scalar1=b_sb[:, m : m + 1]
        )

    def cat_segs(wT, hT, xp_t, nb, cc, ms):
        """Segments for concat(h_own[3 chunks], xp[3 chunks]) for chain cc of
        block nb: pairs (0,1)/(4,5) ride DoubleRow, 2/3 go single (they
        straddle the own/partner tile boundary)."""
        ob = slice(256 * nb, 256 * (nb + 1))
        pb = slice(256 * cc, 256 * (cc + 1))
        return [
            (wT[:, 0:2, ms], hT[cc][:, 0:2, ob], DR),
            (wT[:, 2, ms], hT[cc][:, 2, ob], None),
            (wT[:, 3, ms], xp_t[nb][:, 0, pb], None),
            (wT[:, 4:6, ms], xp_t[nb][:, 1:3, pb], DR),
        ]

    def g1_mgroup(G_t, hT, xp_t, wih, gps, nb, m, cc):
        """One (gate-chunk m, chain cc) piece of G1 block nb."""
        ms = slice(128 * m, 128 * (m + 1))
        ps = gps.tile([128, 256], F32, tag=f"gps{cc}")
        segs = cat_segs(wih, hT, xp_t, nb, cc, ms)
        for si, (lh, rhs, perf) in enumerate(segs):
            nc.tensor.matmul(
                out=ps[:], lhsT=lh, rhs=rhs,
                start=(si == 0), stop=(si == len(segs) - 1), perf_mode=perf,
            )
        gv8 = G_t[nb][:, m, :].rearrange("p (t b) -> p t b", b=GB)
        nc.vector.tensor_scalar_add(
            out=gv8[:, :, 4 * cc : 4 * cc + 4],
            in0=ps[:].rearrange("p (t b) -> p t b", b=SCH),
            scalar1=b_sb[:, MCH + m : MCH + m + 1],
        )

    def recurrence(l, G_t, hT, hTr, whh, on_step=None):
        """LSTM scan over time (t-major token layout); writes hT (pair of
        per-chain tiles, cols (t, b4)) and hTr (pair of per-chain lists of
        per-chunk fp8 tiles, time-reversed).

        Gates in device order (i,f,o,g) with g pre-scaled x2: one Sigmoid
        covers every gate (tanh(x) = 2*sigmoid(2x)-1).  G_t is injected into
        the gate PSUM by an identity matmul; one accumulation group per tile
        (start=True zeroes the whole 2KB bank region).

        The two chains use SEPARATE h tiles: the tile framework tracks deps
        at tile granularity, so a shared tile would make chain 0's matmul
        reads wait on chain 1's h writes and serialize the stagger."""
        Gv8 = [g[:].rearrange("p m (t b) -> p m t b", b=GB) for g in G_t]
        hvs = [h[:].rearrange("p c (t b) -> p c t b", b=SCH) for h in hT]
        with tc.tile_pool(name=f"r{l}", bufs=8) as rp, tc.tile_pool(
            name=f"r{l}c", bufs=1) as rcp, tc.tile_pool(
            name=f"r{l}ps", bufs=2, space="PSUM"
        ) as rps:
            nc.tensor.ldweights(weights=whh[:, 0, 0:1])
            SC = SCH  # two sliding chains of 4 sequences
            c_prev = []
            for cc in range(2):
                cz = rcp.tile([128, KCH, SC], F32, name=f"c0_{l}_{cc}")
                nc.vector.memset(cz[:], 0.0)
                c_prev.append(cz)
            # per-step emission is grouped by engine stage (both chains
            # adjacent) so each engine's in-order queue never has an
            # instruction whose wait blocks the other chain's work.
            for t in range(T):
                gv = Gv8[t // 64][:, :, t % 64, :]  # [128, MCH, 8]
                ps = []
                for cc in range(2):
                    bs = slice(SC * cc, SC * (cc + 1))
                    p = rps.tile([128, MCH, SC], F32, tag=f"ps{cc}")
                    ps.append(p)
                    if t == 0:
                        nc.tensor.matmul(
                            out=p[:], lhsT=ident_bf[:], rhs=gv[:, :, bs],
                            start=True, stop=True, skip_group_check=True,
                        )
                    else:
                        # G-inject first: it has no h dependency, so it soaks
                        # up the stale PSUM-reuse wait and the first whh
                        # matmul carries only the live h wait.
                        nc.tensor.matmul(
                            out=p[:], lhsT=ident_bf[:], rhs=gv[:, :, bs],
                            start=True, stop=False, skip_group_check=True,
                        )
                        for m in range(MCH):
                            for kk in range(KCH):
                                nc.tensor.matmul(
                                    out=p[:, m, :],
                                    lhsT=whh[:, kk, 128 * m : 128 * (m + 1)],
                                    rhs=hvs[cc][:, kk, t - 1, :],
                                    start=False,
                                    stop=(kk == KCH - 1 and m == MCH - 1),
                                    skip_group_check=True,
                                )
                sg = []
                for cc in range(2):
                    s = rp.tile([128, MCH, SC], F32, tag=f"sg{cc}")
                    nc.scalar.activation(out=s[:], in_=ps[cc][:], func=AF.Sigmoid)
                    sg.append(s)
                t1s, t2s = [], []
                for cc in range(2):
                    # c = f*c_prev + i*(2*sg_g - 1); t2p/c fused via stt
                    t1 = rp.tile([128, KCH, SC], F32, tag=f"t1{cc}")
                    nc.vector.tensor_tensor(
                        out=t1[:], in0=sg[cc][:, 3:6, :], in1=c_prev[cc][:], op=ALU.mult
                    )
                    t1s.append(t1)
                for cc in range(2):
                    t2p = rp.tile([128, KCH, SC], F32, tag=f"t2{cc}")
                    nc.vector.scalar_tensor_tensor(
                        out=t2p[:], in0=sg[cc][:, 9:12, :], scalar=0.5,
                        in1=sg[cc][:, 0:3, :], op0=ALU.subtract, op1=ALU.mult,
                    )
                    t2s.append(t2p)
                c_new = []
                for cc in range(2):
                    cn = rp.tile([128, KCH, SC], F32, tag=f"c{cc}")
                    nc.vector.scalar_tensor_tensor(
                        out=cn[:], in0=t2s[cc][:], scalar=2.0, in1=t1s[cc][:],
                        op0=ALU.mult, op1=ALU.add,
                    )
                    c_new.append(cn)
                tc_t = []
                for cc in range(2):
                    tct = rp.tile([128, KCH, SC], F32, tag=f"tc{cc}")
                    nc.scalar.activation(out=tct[:], in_=c_new[cc][:], func=AF.Tanh)
                    tc_t.append(tct)
                for cc in range(2):
                    nc.vector.tensor_tensor(
                        out=hvs[cc][:, :, t, :], in0=sg[cc][:, 6:9, :], in1=tc_t[cc][:], op=ALU.mult
                    )
                rc = T - 1 - t
                for cc in range(2):
                    nc.vector.tensor_tensor(
                        out=hTr[cc][rc // QT][:, :, rc % QT, :],
                        in0=sg[cc][:, 6:9, :], in1=tc_t[cc][:], op=ALU.mult
                    )
                    c_prev[cc] = c_new[cc]
                if on_step is not None:
                    on_step(t)

    def exchange_chunk(l, j, hTr, xp_t):
        """Ship reversed-h chunk j (both chains) to the partner and gather
        the partner's chunk straight into xp_t[j] (cols = (chain, t~, b4),
        matching the own-h layout).  Gather outs must be contiguous —
        strided indirect-DMA outs write garbage (HW-verified)."""
        hq = QN // 2
        ct = ctrb[l][j].ap().rearrange("c p n -> p c n")
        nc.sync.dma_start(out=ct[:, :, 0:hq], in_=hTr[0][j][:])
        nc.sync.dma_start(out=ct[:, :, hq:QN], in_=hTr[1][j][:])
        nc.gpsimd.collective_compute(
            "AllGather",
            ALU.bypass,
            replica_groups=PAIRS,
            ins=[ctrb[l][j].ap()],
            outs=[hall[l][j].ap()],
        )
        rows = hall[l][j].ap().rearrange("r c p n -> (r c p) n")
        for cch in range(KCH):
            nc.gpsimd.indirect_dma_start(
                out=xp_t[j][:, cch, :],
                out_offset=None,
                in_=rows,
                in_offset=IndirectOffsetOnAxis(ap=gidx_sb[:, cch : cch + 1], axis=0),
            )

    def make_on_step(l, hTr, xp_t, work=None):
        work = list(work) if work else []

        def on_step(t):
            # up to one queued emission (embed ktile / G0 m-group) per step,
            # woven into the recurrence's engine slack
            if work and t % 2 == 0:
                work.pop(0)()
            # chunk j's reversed cols are complete after step T-1-QT*j
            if t >= QT - 1 and (t + 1) % QT == 0 and t != T - 1:
                j = (T - 1 - t) // QT
                exchange_chunk(l, j, hTr, xp_t)
            if t == T - 1:
                while work:
                    work.pop(0)()

        return on_step

    def exchange_last(l, hTr, xp_t):
        exchange_chunk(l, 0, hTr, xp_t)
        for cch in range(KCH):
            nc.tensor.ldweights(weights=xp_t[0][:, cch, 0:1])

    # ---- layer pipeline with scoped lifetimes (strict LIFO pools) ----
    with tc.tile_pool(name="phh", bufs=1) as phh:
        half = NT // 2
        hT0 = [phh.tile([128, KCH, half], F8E4, name=f"hT0{c}") for c in "ab"]
        hT1 = [phh.tile([128, KCH, half], F8E4, name=f"hT1{c}") for c in "ab"]
        hTr0 = [
            [phh.tile([128, KCH, QT, SCH], F8E4, name=f"hTr0{c}_{j}") for j in range(NCHUNK)]
            for c in "ab"
        ]
        hTr1 = [
            [phh.tile([128, KCH, QT, SCH], F8E4, name=f"hTr1{c}_{j}") for j in range(NCHUNK)]
            for c in "ab"
        ]
        xp_t = [phh.tile([128, KCH, QN], F8E4, name=f"xp{j}") for j in range(NCHUNK)]
        with tc.tile_pool(name="pg", bufs=1) as pgp:
            G_t = [pgp.tile([128, MCH, 512], BF16, name=f"G{nb}") for nb in range(NBG)]
            with tc.tile_pool(name="pr0", bufs=1) as pr0:
                whh0 = load_whh(0, pr0)
                with tc.tile_pool(name="pw0", bufs=1) as pw0:
                    wih0 = load_wih(0, pw0)
                    with tc.tile_pool(name="px", bufs=1) as px, tc.tile_pool(
                        name="s1", bufs=2
                    ) as s1, tc.tile_pool(
                        name="s1ps", bufs=1, space="PSUM"
                    ) as s1ps, tc.tile_pool(
                        name="g0ps", bufs=1, space="PSUM"
                    ) as g0ps:
                        xT_t = [
                            px.tile([128, DCH, 512], F8E4, name=f"xT{nb}")
                            for nb in range(NBG)
                        ]
                        embed_ktile = make_embed_ktile(s1, s1ps, xT_t)
                        # block 0 up front (the recurrence needs it at step 0);
                        # the rest weaves into the recurrence's engine slack
                        for k in range(4):
                            embed_ktile(k)
                        for m in range(MCH):
                            g0_mgroup(G_t, xT_t, wih0, g0ps, 0, m)
                        work = []
                        for nb in range(1, NBG):
                            for k in range(4 * nb, 4 * nb + 4):
                                work.append(lambda k=k: embed_ktile(k))
                            for m in range(0, MCH, 3):
                                work.append(
                                    lambda nb=nb, m=m: [
                                        g0_mgroup(G_t, xT_t, wih0, g0ps, nb, mm)
                                        for mm in range(m, m + 3)
                                    ]
                                )
                        recurrence(
                            0, G_t, hT0, hTr0, whh0,
                            on_step=make_on_step(0, hTr0, xp_t, work=work),
                        )
            exchange_last(0, hTr0, xp_t)
            with tc.tile_pool(name="pw1", bufs=1) as pw1, tc.tile_pool(
                name="g1ps", bufs=2, space="PSUM"
            ) as g1ps:
                wih1 = load_wih(1, pw1)
                nc.tensor.ldweights(weights=wih1[:, 0, 0:1])
                for nb in (1, 2, 3, 0):
                    for m in range(MCH):
                        for cc in range(2):
                            g1_mgroup(G_t, hT0, xp_t, wih1, g1ps, nb, m, cc)
            with tc.tile_pool(name="pr1", bufs=1) as pr1:
                whh1 = load_whh(1, pr1)
                recurrence(
                    1, G_t, hT1, hTr1, whh1,
                    on_step=make_on_step(1, hTr1, xp_t),
                )
        exchange_last(1, hTr1, xp_t)

        # ---- emissions: em^T [C, NT] = fc @ concat(h2_own, h2_partner) ----
        crf_cm = tc.tile_pool(name="crf", bufs=1)
        crf = crf_cm.__enter__()
        labf_sb = crf.tile([1, NT], F32, name="labf_sb")
        nc.sync.dma_start(out=labf_sb[:], in_=ins["labf"])
        pe_touch_f32(cpack_sb[:, 0:1])
        tileA = crf.tile([C, NT], F32, name="tileA")  # emT, later M1/pd
        tileB = crf.tile([C, NT], F32, name="tileB")  # Q
        tileC = crf.tile([C, NT], F32, name="tileC")  # lab_bc, later gem
        tileD = crf.tile([C, NT], F32, name="tileD")  # OH
        emT = tileA
        with tc.tile_pool(name="emps", bufs=2, space="PSUM") as emps:
            nc.tensor.ldweights(weights=fcT_sb[:, 0, 0:1])
            emv8 = emT[:].rearrange("c (t b) -> c t b", b=GB)
            for nb in (1, 2, 3, 0):
                for cc in range(2):
                    ps = emps.tile([C, 256], F32, tag=f"emps{cc}")
                    # no DoubleRow: fcT's k-pair stride (C=14B) violates the
                    # dual-fp8 LDWEIGHTS stride%16 ISA restriction
                    ob = slice(256 * nb, 256 * (nb + 1))
                    pb = slice(256 * cc, 256 * (cc + 1))
                    for kk in range(DCH):
                        rhs = (
                            hT1[cc][:, kk, ob]
                            if kk < KCH
                            else xp_t[nb][:, kk - KCH, pb]
                        )
                        nc.tensor.matmul(
                            out=ps[:],
                            lhsT=fcT_sb[:, kk, :],
                            rhs=rhs,
                            start=(kk == 0),
                            stop=(kk == DCH - 1),
                        )
                    nc.vector.tensor_scalar_add(
                        out=emv8[:, 64 * nb : 64 * (nb + 1), 4 * cc : 4 * cc + 4],
                        in0=ps[:].rearrange("c (t b) -> c t b", b=SCH),
                        scalar1=fcb_sb[:],
                    )
        if DEBUG_OUTS:
            nc.sync.dma_start(out=dbg["dbg_em"], in_=emT[:])

        # ---- CRF ----
        with tc.tile_pool(name="crfw", bufs=4) as cw, tc.tile_pool(
            name="crfps", bufs=1, space="PSUM"
        ) as cps:
            Q = tileB
            nc.scalar.activation(out=Q[:], in_=emT[:], func=AF.Exp)
            dve_touch(Q[0:1, 0:1])
            Qv = Q[:].rearrange("c (t b) -> c t b", b=GB)

            # labels broadcast across the C partitions
            lab_bc = tileC
            for nb in range(NT // 512):
                bps = cps.tile([C, 512], F32, tag="cps512")
                nc.tensor.matmul(
                    out=bps[:],
                    lhsT=ones1C[:],
                    rhs=labf_sb[:, 512 * nb : 512 * (nb + 1)],
                    start=True,
                    stop=True,
                )
                nc.vector.tensor_copy(out=lab_bc[:, 512 * nb : 512 * (nb + 1)], in_=bps[:])
            OH = tileD
            nc.vector.tensor_scalar(
                out=OH[:], in0=lab_bc[:], scalar1=iota_sb[:], scalar2=None, op0=ALU.is_equal
            )

            # gold emissions total (gem overwrites lab_bc slot)
            gem = tileC
            nc.vector.tensor_tensor(out=gem[:], in0=emT[:], in1=OH[:], op=ALU.mult)
            gem_r = cw.tile([C, 1], F32, tag="gred")
            nc.vector.reduce_sum(out=gem_r[:], in_=gem[:], axis=mybir.AxisListType.X)

            # transition pairs: M1 = trans @ OH_next (t-major: the successor
            # token is 8 columns over; the last 8 cols have no successor)
            M1 = tileA
            for nb in range(NT // 512):
                lo = 512 * nb
                hi = min(512 * (nb + 1), NT - 8)
                mps = cps.tile([C, 512], F32, tag="cps512")
                nc.tensor.matmul(
                    out=mps[:, : hi - lo],
                    lhsT=transT_sb[:],
                    rhs=OH[:, lo + 8 : hi + 8],
                    start=True,
                    stop=True,
                )
                nc.vector.tensor_copy(out=M1[:, lo:hi], in_=mps[:, : hi - lo])
            nc.vector.tensor_tensor(
                out=M1[:, : NT - 8], in0=OH[:, : NT - 8], in1=M1[:, : NT - 8], op=ALU.mult
            )
            pd_r = cw.tile([C, 1], F32, tag="pdr")
            nc.vector.reduce_sum(out=pd_r[:], in_=M1[:, : NT - 8], axis=mybir.AxisListType.X)

            # start/end terms (first/last 8 columns in t-major)
            st8 = cw.tile([C, GB], F32, tag="st8")
            nc.vector.tensor_scalar_mul(out=st8[:], in0=OH[:, 0:GB], scalar1=stv_sb[:])
            st_r = cw.tile([C, 1], F32, tag="str")
            nc.vector.reduce_sum(out=st_r[:], in_=st8[:], axis=mybir.AxisListType.X)
            en8 = cw.tile([C, GB], F32, tag="en8")
            nc.vector.tensor_scalar_mul(out=en8[:], in0=OH[:, NT - GB : NT], scalar1=env_sb[:])
            en_r = cw.tile([C, 1], F32, tag="enr")
            nc.vector.reduce_sum(out=en_r[:], in_=en8[:], axis=mybir.AxisListType.X)

            score_ps = cps.tile([1, 8], F32, tag="scoreps")
            for i, r in enumerate((gem_r, pd_r, st_r, en_r)):
                nc.tensor.matmul(
                    out=score_ps[:1, :1],
                    lhsT=onesC1[:],
                    rhs=r[:],
                    start=(i == 0),
                    stop=(i == 3),
                    skip_group_check=True,
                )
            score_sb = cw.tile([1, 1], F32, tag="scoresb")
            nc.vector.tensor_copy(out=score_sb[:], in_=score_ps[:1, :1])

            # ---- blocked forward chain in exp domain, E pre-scaled by
            # 1/ESCALE so no renorms are needed (f32 range absorbs the
            # drift; the log-correction is added at the end).
            # Block 0 runs the alpha chain over t in [0, BT); blocks k=1..3
            # run transfer-matrix chains Yk = M_k^T = E'D_lo ... E'D_hi
            # (built high-t to low-t), all NBLK chains concurrently.
            expTTp_bf = cw.tile([C, C], BF16, tag="ettbf")
            nc.vector.tensor_copy(out=expTTp_bf[:], in_=expTTp_f32)
            v_prev = cw.tile([C, GB], F32, tag="v")
            nc.vector.tensor_scalar_mul(out=v_prev[:], in0=Qv[:, 0, :], scalar1=expst_sb[:])
            eye_bc = eye_sb.unsqueeze(1).broadcast_to([C, GB, C])
            yps_prev = [None] * NBLK
            for s in range(BT):
                if s > 0:
                    vps = cps.tile([C, GB], F32, tag="vps")
                    nc.tensor.matmul(out=vps[:], lhsT=E_sb[:], rhs=v_prev[:], start=True, stop=True)
                    v_new = cw.tile([C, GB], F32, tag="v")
                    nc.vector.tensor_tensor(out=v_new[:], in0=vps[:], in1=Qv[:, s, :], op=ALU.mult)
                    v_prev = v_new
                for k in range(1, NBLK):
                    t = BT * (k + 1) - 1 - s
                    qb = Qv[:, t, :].unsqueeze(2).broadcast_to([C, GB, C])
                    w = cw.tile([C, GB, C], BF16, tag=f"w{k}")
                    if s == 0:
                        nc.vector.tensor_tensor(out=w[:], in0=eye_bc, in1=qb, op=ALU.mult)
                    else:
                        nc.vector.tensor_tensor(
                            out=w[:],
                            in0=yps_prev[k][:].rearrange("c (b j) -> c b j", b=GB),
                            in1=qb,
                            op=ALU.mult,
                        )
                    yp = cps.tile([C, GB * C], F32, tag=f"yps{k}")
                    nc.tensor.matmul(out=yp[:], lhsT=expTTp_bf[:], rhs=w[:], start=True, stop=True)
                    yps_prev[k] = yp
            yfin = []
            for k in range(1, NBLK):
                yf = cw.tile([C, GB, C], BF16, tag=f"yf{k}")
                nc.vector.tensor_copy(
                    out=yf[:], in_=yps_prev[k][:].rearrange("c (b j) -> c b j", b=GB)
                )
                yfin.append(yf)
            # fold block results into the alpha vector
            for k in range(1, NBLK):
                vb = cw.tile([C, GB], BF16, tag="vb")
                nc.vector.tensor_copy(out=vb[:], in_=v_prev[:])
                aps = cps.tile([C, GB], F32, tag="vps")
                for b in range(GB):
                    nc.tensor.matmul(
                        out=aps[:, b : b + 1],
                        lhsT=yfin[k - 1][:, b, :],
                        rhs=vb[:, b : b + 1],
                        start=(b == 0),
                        stop=(b == GB - 1),
                        skip_group_check=True,
                    )
                v_new = cw.tile([C, GB], F32, tag="v")
                nc.vector.tensor_copy(out=v_new[:], in_=aps[:])
                v_prev = v_new
            vend = cw.tile([C, GB], F32, tag="vend")
            nc.vector.tensor_scalar_mul(out=vend[:], in0=v_prev[:], scalar1=expen_sb[:])
            zps = cps.tile([1, GB], F32, tag="cps1")
            nc.tensor.matmul(out=zps[:], lhsT=onesC1[:], rhs=vend[:], start=True, stop=True)
            lnz = cw.tile([1, GB], F32, tag="lnz")
            nc.scalar.activation(out=lnz[:], in_=zps[:], func=AF.Ln)
            logz = cw.tile([1, GB], F32, tag="logz")
            nc.vector.tensor_scalar(
                out=logz[:], in0=lnz[:], scalar1=float((T - 1) * np.log(ESCALE)),
                scalar2=None, op0=ALU.add,
            )
            lz_tot = cw.tile([1, 1], F32, tag="lztot")
            nc.vector.reduce_sum(out=lz_tot[:], in_=logz[:], axis=mybir.AxisListType.X)
            loss_sb = cw.tile([1, 1], F32, tag="loss_sb")
            nc.vector.tensor_tensor(out=loss_sb[:], in0=lz_tot[:], in1=score_sb[:], op=ALU.subtract)
            nc.sync.dma_start(out=loss_out, in_=loss_sb[:])
            if DEBUG_OUTS:
                dsc = cw.tile([1, 2], F32, tag="dsc")
                nc.vector.tensor_copy(out=dsc[:, 0:1], in_=lz_tot[:])
                nc.vector.tensor_copy(out=dsc[:, 1:2], in_=score_sb[:])
                nc.sync.dma_start(out=dbg["dbg_sc"], in_=dsc[:])
        crf_cm.__exit__(None, None, None)

    est.close()


# ---------------------------------------------------------------------------
# host side
# ---------------------------------------------------------------------------

def make_in_maps(inputs):
    ids = np.asarray(inputs["input_ids"]).astype(np.int64)
    labels = np.asarray(inputs["labels"]).astype(np.int64)
    word_emb = _f32(inputs["word_emb"])
    pos_emb = _f32(inputs["pos_emb"])
    type_emb = _f32(inputs["type_emb"])
    ln_g = _f32(inputs["ln_g"])
    ln_b = _f32(inputs["ln_b"])
    w_ih = _f32(inputs["w_ih"])
    w_hh = _f32(inputs["w_hh"])
    b_ih = _f32(inputs["b_ih"])
    b_hh = _f32(inputs["b_hh"])
    fc_w = _f32(inputs["fc_w"])
    fc_b = _f32(inputs["fc_b"])
    crf_start = _f32(inputs["crf_start"])
    crf_end = _f32(inputs["crf_end"])
    crf_trans = _f32(inputs["crf_trans"])

    posty0 = pos_emb[:T] + type_emb[0][None, :]
    word_emb_bf = _bf(word_emb)

    in_maps = []
    for core in range(NCORES):
        g, d = core // 2, core % 2
        sl = slice(GB * g, GB * (g + 1))
        ids_loc = ids[sl]
        lab_loc = labels[sl]
        posty = posty0
        if d == 1:
            ids_loc = ids_loc[:, ::-1]
            lab_loc = lab_loc[:, ::-1]
            posty = posty0[::-1]

        # layer-0 weights with LN affine folded in (gate-permuted, g x2)
        w0 = _perm_gates(w_ih[0, d] * ln_g[None, :])
        bias0 = _perm_gates((b_ih[0, d] + b_hh[0, d] + w_ih[0, d] @ ln_b)[:, None])[:, 0]
        # layer-1 weights, columns permuted to local [own, partner] order
        w1 = w_ih[1, d]
        if d == 1:
            w1 = np.concatenate([w1[:, HD:], w1[:, :HD]], axis=1)
        w1 = _perm_gates(w1)
        bias1 = _perm_gates((b_ih[1, d] + b_hh[1, d])[:, None])[:, 0]
        fcp = fc_w if d == 0 else np.concatenate([fc_w[:, HD:], fc_w[:, :HD]], axis=1)

        trans_eff = crf_trans if d == 0 else crf_trans.T
        start_eff = crf_start if d == 0 else crf_end
        end_eff = crf_end if d == 0 else crf_start

        pr = 1 - d
        gidx = np.empty((128, KCH), np.int32)
        for cch in range(KCH):
            gidx[:, cch] = pr * (KCH * 128) + cch * 128 + np.arange(128)

        cpack = np.zeros((C, CPW), np.float32)
        cpack[:, 0:C] = np.exp(trans_eff) / ESCALE
        cpack[:, C : 2 * C] = trans_eff.T
        cpack[:, 28] = np.exp(start_eff)
        cpack[:, 29] = np.exp(end_eff)
        cpack[:, 30] = start_eff
        cpack[:, 31] = end_eff
        cpack[:, 32] = np.arange(C, dtype=np.float32)
        cpack[:, 33] = fc_b
        cpack[:, 34 : 34 + C] = (np.exp(trans_eff) / ESCALE).T
        cpack[:, 34 + C : 34 + 2 * C] = np.eye(C, dtype=np.float32)

        b01 = np.concatenate(
            [bias0.reshape(MCH, 128).T, bias1.reshape(MCH, 128).T], axis=1
        )

        # t-major token order: token n = (t, b)
        ids_tm = np.ascontiguousarray(ids_loc.T).reshape(NT, 1)
        lab_tm = np.ascontiguousarray(lab_loc.T).reshape(1, NT)
        posty_rep = np.repeat(np.asarray(posty, np.float32), GB, axis=0)
        in_maps.append(
            dict(
                ids32=np.ascontiguousarray(ids_tm.astype(np.int32)),
                labf=np.ascontiguousarray(lab_tm.astype(np.float32)),
                word_emb=word_emb_bf,
                posty=_bf(posty_rep),
                wih0T=_f8(w0.T),
                wih1T=_f8(w1.T),
                whh0T=_bf(_perm_gates(w_hh[0, d]).T),
                whh1T=_bf(_perm_gates(w_hh[1, d]).T),
                b01=np.ascontiguousarray(b01),
                fcT=_f8(fcp.T),
                cpack=cpack,
                gidx=gidx,
            )
        )
    return in_maps


_PROGRAM = None
_COST_MODEL_NS = None


def _get_program():
    global _PROGRAM, _COST_MODEL_NS
    if _PROGRAM is None:
        _PROGRAM = build_program()
        try:
            from concourse.timeline_sim import TimelineSim

            _COST_MODEL_NS = int(TimelineSim(_PROGRAM, trace=False, no_exec=True).simulate())
        except Exception:
            _COST_MODEL_NS = None
    return _PROGRAM


def run(inputs, trace=False):
    nc = _get_program()
    in_maps = make_in_maps(inputs)
    res = run_bass_kernel_spmd(nc, in_maps, core_ids=list(range(NCORES)), trace=trace)
    total = np.float64(0.0)
    for g in range(4):
        total += np.float64(res.results[2 * g]["loss"][0, 0])
    return np.asarray(total, dtype=np.float32), res


def kernel(**inputs):
    out, _ = run(inputs, trace=False)
    return out



# revision 66
# speedup vs baseline: 1.0056x; 1.0056x over previous
"""BiLSTM-CRF sequence-tagging loss on 8 Trainium2 NeuronCores.

Sharding: 8 cores = 4 batch-groups x 2 LSTM directions.
  core 2g+d handles sequences [8g, 8g+8) ; d=0 forward, d=1 backward.
Backward cores receive time-reversed inputs (ids/pos/labels), so one SPMD
program runs on all cores; their CRF uses transposed transitions with
start/end swapped (same loss by path reversal), and their layer-2/emission
weights are column-permuted so the local [own_h, partner_h] concat order is
uniform.  The h-streams are exchanged pairwise via AllGather through DRAM;
the partner slot is fetched with an indirect-DMA row gather whose indices
are per-core input data (keeps the program core-uniform).
"""

import os
import sys

import numpy as np

for _p in ("/opt/trn_rl_repo", "/root/.axon_site/_ro/trn_rl_repo"):
    if os.path.isdir(_p) and _p not in sys.path:
        sys.path.insert(0, _p)

import ml_dtypes  # noqa: E402

import concourse.bass as bass  # noqa: E402
import concourse.bacc as bacc  # noqa: E402
import concourse.tile as tile  # noqa: E402
from concourse import mybir  # noqa: E402
from concourse.bass import IndirectOffsetOnAxis  # noqa: E402
from concourse.bass_utils import run_bass_kernel_spmd  # noqa: E402
from concourse.masks import make_identity  # noqa: E402

F32 = mybir.dt.float32
BF16 = mybir.dt.bfloat16
F8E4 = mybir.dt.float8e4
I32 = mybir.dt.int32
AF = mybir.ActivationFunctionType
ALU = mybir.AluOpType

# problem shapes (hardcoded per contract)
B, T, V, D, C, HD = 32, 256, 30522, 768, 14, 384
L = 2
NCORES = 8
GB = 8            # sequences per core group
NT = GB * T       # tokens per core = 2048
NTILE = NT // 128  # 16
MCH = 12          # gate chunks of 128 (4*HD/128)
KCH = 3           # hidden chunks (HD/128)
DCH = 6           # input-dim chunks (D/128)
LN_EPS = 1e-12
PAIRS = [[0, 1], [2, 3], [4, 5], [6, 7]]
ESCALE = 16.0     # folded per-step scaling of exp(trans); no renorms needed
NBLK = 4          # CRF scan blocks (1 alpha chain + NBLK-1 matrix chains)
BT = T // NBLK    # 64 steps per block
CPW = 34 + 2 * 14  # cpack width
NCHUNK = 4        # exchange chunks per layer
QT = T // NCHUNK  # 64 timesteps per chunk
QN = GB * QT      # 512 columns per chunk
SCH = GB // 2     # sequences per recurrence chain
NBG = NT // 512   # 512-token (64-step) blocks

DEBUG_OUTS = False


def _bf(x):
    return np.ascontiguousarray(np.asarray(x, dtype=np.float32)).astype(ml_dtypes.bfloat16)


def _f32(x):
    return np.ascontiguousarray(np.asarray(x, dtype=np.float32))


def _f8(x):
    return np.ascontiguousarray(np.asarray(x, dtype=np.float32)).astype(
        ml_dtypes.float8_e4m3
    )


def _perm_gates(w):
    """torch gate order i,f,g,o -> device order i,f,o,g with the g block
    scaled by 2 (tanh(x) = 2*sigmoid(2x)-1)."""
    H = HD
    return np.concatenate([w[0:H], w[H:2 * H], w[3 * H:4 * H], 2.0 * w[2 * H:3 * H]], axis=0)


# ---------------------------------------------------------------------------
# device program
# ---------------------------------------------------------------------------

def build_program():
    nc = bacc.Bacc("TRN2", target_bir_lowering=False, debug=False, num_devices=NCORES)

    def din(name, shape, dt):
        return nc.dram_tensor(name, shape, dt, kind="ExternalInput").ap()

    ins = dict(
        ids32=din("ids32", [NT, 1], I32),
        labf=din("labf", [1, NT], F32),
        word_emb=din("word_emb", [V, D], BF16),
        posty=din("posty", [NT, D], BF16),
        wih0T=din("wih0T", [D, 4 * HD], F8E4),
        wih1T=din("wih1T", [D, 4 * HD], F8E4),
        whh0T=din("whh0T", [HD, 4 * HD], BF16),
        whh1T=din("whh1T", [HD, 4 * HD], BF16),
        b01=din("b01", [128, 2 * MCH], F32),
        fcT=din("fcT", [D, C], F8E4),
        cpack=din("cpack", [C, CPW], F32),
        gidx=din("gidx", [128, KCH], I32),
    )

    loss_out = nc.dram_tensor("loss", [1, 1], F32, kind="ExternalOutput").ap()
    dbg = {}
    if DEBUG_OUTS:
        dbg["dbg_xt"] = nc.dram_tensor("dbg_xt", [128, DCH, NT], BF16, kind="ExternalOutput").ap()
        dbg["dbg_g"] = nc.dram_tensor("dbg_g", [128, MCH, NT], BF16, kind="ExternalOutput").ap()
        dbg["dbg_h1"] = nc.dram_tensor("dbg_h1", [128, KCH, NT], BF16, kind="ExternalOutput").ap()
        dbg["dbg_h2"] = nc.dram_tensor("dbg_h2", [128, KCH, NT], BF16, kind="ExternalOutput").ap()
        dbg["dbg_em"] = nc.dram_tensor("dbg_em", [C, NT], F32, kind="ExternalOutput").ap()
        dbg["dbg_sc"] = nc.dram_tensor("dbg_sc", [1, 2], F32, kind="ExternalOutput").ap()

    # internal DRAM for pairwise exchange (fp8, 4 time-chunks per layer so
    # collectives overlap the recurrence)
    ctrb = [
        [nc.dram_tensor(f"ctrb{l}_{j}", [KCH, 128, QN], F8E4) for j in range(NCHUNK)]
        for l in range(L)
    ]
    hall = [
        [nc.dram_tensor(f"hall{l}_{j}", [2, KCH, 128, QN], F8E4) for j in range(NCHUNK)]
        for l in range(L)
    ]

    with tile.TileContext(nc) as tc:
        _build_body(tc, ins, loss_out, dbg, ctrb, hall)

    nc.compile()
    return nc


def _build_body(tc, ins, loss_out, dbg, ctrb, hall):
    nc = tc.nc
    from contextlib import ExitStack

    est = ExitStack()
    pers = est.enter_context(tc.tile_pool(name="pers", bufs=1))

    # ---- persistent SBUF state (small constants only) ----
    def load_wih(l, pool):
        wt = pool.tile([128, DCH, 4 * HD], F8E4, name=f"wih{l}")
        src = ins["wih0T"] if l == 0 else ins["wih1T"]
        nc.sync.dma_start(out=wt[:], in_=src.rearrange("(k p) m -> p k m", p=128))
        return wt

    def load_whh(l, pool):
        ht = pool.tile([128, KCH, 4 * HD], BF16, name=f"whh{l}")
        src = ins["whh0T"] if l == 0 else ins["whh1T"]
        nc.sync.dma_start(out=ht[:], in_=src.rearrange("(k p) m -> p k m", p=128))
        return ht

    # scratch + absorbers: this toolchain allows only ONE sem wait per
    # instruction, so every junction of two producers gets a tiny absorber op
    # that folds one producer into the consuming engine's clock first.
    scr_dve = pers.tile([1, 4], F32, name="scr_dve")
    scr_gp = pers.tile([1, 4], I32, name="scr_gp")
    pabs = est.enter_context(tc.tile_pool(name="pabs", bufs=1, space="PSUM"))
    pscr = pabs.tile([1, 8], F32, name="pscr")

    def dve_touch(ap):
        nc.vector.tensor_copy(out=scr_dve[:, 0:1], in_=ap)

    def pe_touch_f32(ap_col):
        nc.tensor.matmul(out=pscr[:1, :1], lhsT=ap_col, rhs=ap_col, start=True, stop=True)

    b_sb = pers.tile([128, 2 * MCH], F32, name="b_sb")
    nc.sync.dma_start(out=b_sb[:], in_=ins["b01"])
    dve_touch(b_sb[0:1, 0:1])

    fcT_sb = pers.tile([128, DCH, C], F8E4, name="fcT")
    nc.sync.dma_start(out=fcT_sb[:], in_=ins["fcT"].rearrange("(k p) m -> p k m", p=128))

    cpack_sb = pers.tile([C, CPW], F32, name="cpack_sb")
    nc.sync.dma_start(out=cpack_sb[:], in_=ins["cpack"])
    dve_touch(cpack_sb[0:1, 0:1])
    E_sb = cpack_sb[:, 0:C]  # exp(trans)/ESCALE
    transT_sb = cpack_sb[:, C : 2 * C]
    expst_sb = cpack_sb[:, 28:29]
    expen_sb = cpack_sb[:, 29:30]
    stv_sb = cpack_sb[:, 30:31]
    env_sb = cpack_sb[:, 31:32]
    iota_sb = cpack_sb[:, 32:33]
    fcb_sb = cpack_sb[:, 33:34]
    expTTp_f32 = cpack_sb[:, 34 : 34 + C]  # exp(trans.T)/ESCALE
    eye_sb = cpack_sb[:, 34 + C : 34 + 2 * C]  # identity

    gidx_sb = pers.tile([128, KCH], I32, name="gidx_sb")
    nc.sync.dma_start(out=gidx_sb[:], in_=ins["gidx"])
    nc.gpsimd.tensor_copy(out=scr_gp[:, 0:1], in_=gidx_sb[0:1, 0:1])

    ids_sb = pers.tile([128, NTILE], I32, name="ids_sb")
    nc.sync.dma_start(out=ids_sb[:], in_=ins["ids32"].rearrange("(k p) o -> p (k o)", p=128))

    ident = pers.tile([128, 128], F32, name="ident")
    make_identity(nc, ident[:])
    ident_bf = pers.tile([128, 128], BF16, name="ident_bf")
    nc.vector.tensor_copy(out=ident_bf[:], in_=ident[:])
    pe_touch_f32(ident[:, 0:1])
    eps_sb = pers.tile([128, 1], F32, name="eps_sb")
    nc.vector.memset(eps_sb[:], LN_EPS)
    ones1C = pers.tile([1, C], F32, name="ones1C")
    nc.vector.memset(ones1C[:], 1.0)
    onesC1 = pers.tile([C, 1], F32, name="onesC1")
    nc.vector.memset(onesC1[:], 1.0)

    # ---- helpers ----
    def make_embed_ktile(s1, s1ps, xT_t):
        """Returns emit(k): embeds token k-tile k (128 t-major tokens) into
        xT_t[k//4]. Called lazily so ktiles can be woven into the layer-0
        recurrence's engine slack."""

        def emit(k):
            posty_sb = s1.tile([128, D], BF16, tag="posty")
            nc.sync.dma_start(
                out=posty_sb[:], in_=ins["posty"][128 * k : 128 * (k + 1), :]
            )
            emb = s1.tile([128, D], BF16, tag="emb")
            nc.gpsimd.indirect_dma_start(
                out=emb[:],
                out_offset=None,
                in_=ins["word_emb"],
                in_offset=IndirectOffsetOnAxis(ap=ids_sb[:, k : k + 1], axis=0),
            )
            emb2 = s1.tile([128, D], BF16, tag="emb2")
            nc.vector.tensor_add(out=emb2[:], in0=emb[:], in1=posty_sb[:])
            stats = s1.tile([128, 3, 6], F32, tag="stats")
            embv = emb2[:].rearrange("p (s q) -> p s q", s=3)
            for sg in range(3):
                nc.vector.bn_stats(out=stats[:, sg, :], in_=embv[:, sg, :])
            mv = s1.tile([128, 2], F32, tag="mv")
            nc.vector.bn_aggr(out=mv[:], in_=stats[:])
            std = s1.tile([128, 1], F32, tag="std")
            nc.scalar.activation(out=std[:], in_=mv[:, 1:2], func=AF.Sqrt, bias=eps_sb[:])
            rstd = s1.tile([128, 1], F32, tag="rstd")
            nc.vector.reciprocal(out=rstd[:], in_=std[:])
            xln = s1.tile([128, D], BF16, tag="xln")
            nc.vector.tensor_scalar(
                out=xln[:],
                in0=emb2[:],
                scalar1=mv[:, 0:1],
                scalar2=rstd[:],
                op0=ALU.subtract,
                op1=ALU.mult,
            )
            for j in range(DCH):
                tp = s1ps.tile([128, 128], BF16, tag="tp")
                nc.tensor.transpose(
                    out=tp[:], in_=xln[:, 128 * j : 128 * (j + 1)], identity=ident_bf[:]
                )
                # spread PSUM->SBUF copies across engines; DVE is the
                # embed bottleneck
                dst = xT_t[k // 4][:, j, 128 * (k % 4) : 128 * (k % 4 + 1)]
                if j % 2 == 0:
                    nc.vector.tensor_copy(out=dst, in_=tp[:])
                else:
                    nc.scalar.copy(out=dst, in_=tp[:])

        return emit

    DR = mybir.MatmulPerfMode.DoubleRow

    def g0_mgroup(G_t, xT_t, wih, gps, nb, m):
        """One gate-chunk m of G0 block nb: 3 fp8 DoubleRow matmuls + bias."""
        ms = slice(128 * m, 128 * (m + 1))
        ps = gps.tile([128, 512], F32, tag="gps")
        for kp in range(DCH // 2):
            nc.tensor.matmul(
                out=ps[:],
                lhsT=wih[:, 2 * kp : 2 * kp + 2, ms],
                rhs=xT_t[nb][:, 2 * kp : 2 * kp + 2, :],
                start=(kp == 0),
                stop=(kp == DCH // 2 - 1),
                perf_mode=DR,
            )
        nc.vector.tensor_scalar_add(
            out=G_t[nb][:, m, :], in0=ps[:], scalar1=b_sb[:, m : m + 1]
        )

    def cat_segs(wT, hT, xp_t, nb, cc, ms):
        """Segments for concat(h_own[3 chunks], xp[3 chunks]) for chain cc of
        block nb: pairs (0,1)/(4,5) ride DoubleRow, 2/3 go single (they
        straddle the own/partner tile boundary)."""
        ob = slice(256 * nb, 256 * (nb + 1))
        pb = slice(256 * cc, 256 * (cc + 1))
        return [
            (wT[:, 0:2, ms], hT[cc][:, 0:2, ob], DR),
            (wT[:, 2, ms], hT[cc][:, 2, ob], None),
            (wT[:, 3, ms], xp_t[nb][:, 0, pb], None),
            (wT[:, 4:6, ms], xp_t[nb][:, 1:3, pb], DR),
        ]

    def g1_mgroup(G_t, hT, xp_t, wih, gps, nb, m, cc):
        """One (gate-chunk m, chain cc) piece of G1 block nb."""
        ms = slice(128 * m, 128 * (m + 1))
        ps = gps.tile([128, 256], F32, tag=f"gps{cc}")
        segs = cat_segs(wih, hT, xp_t, nb, cc, ms)
        for si, (lh, rhs, perf) in enumerate(segs):
            nc.tensor.matmul(
                out=ps[:], lhsT=lh, rhs=rhs,
                start=(si == 0), stop=(si == len(segs) - 1), perf_mode=perf,
            )
        gv8 = G_t[nb][:, m, :].rearrange("p (t b) -> p t b", b=GB)
        nc.vector.tensor_scalar_add(
            out=gv8[:, :, 4 * cc : 4 * cc + 4],
            in0=ps[:].rearrange("p (t b) -> p t b", b=SCH),
            scalar1=b_sb[:, MCH + m : MCH + m + 1],
        )

    def recurrence(l, G_t, hT, hTr, whh, on_step=None):
        """LSTM scan over time (t-major token layout); writes hT (pair of
        per-chain tiles, cols (t, b4)) and hTr (pair of per-chain lists of
        per-chunk fp8 tiles, time-reversed).

        Gates in device order (i,f,o,g) with g pre-scaled x2: one Sigmoid
        covers every gate (tanh(x) = 2*sigmoid(2x)-1).  G_t is injected into
        the gate PSUM by an identity matmul; one accumulation group per tile
        (start=True zeroes the whole 2KB bank region).

        The two chains use SEPARATE h tiles: the tile framework tracks deps
        at tile granularity, so a shared tile would make chain 0's matmul
        reads wait on chain 1's h writes and serialize the stagger."""
        Gv8 = [g[:].rearrange("p m (t b) -> p m t b", b=GB) for g in G_t]
        hvs = [h[:].rearrange("p c (t b) -> p c t b", b=SCH) for h in hT]
        with tc.tile_pool(name=f"r{l}", bufs=8) as rp, tc.tile_pool(
            name=f"r{l}c", bufs=1) as rcp, tc.tile_pool(
            name=f"r{l}ps", bufs=2, space="PSUM"
        ) as rps:
            nc.tensor.ldweights(weights=whh[:, 0, 0:1])
            SC = SCH  # two sliding chains of 4 sequences
            c_prev = []
            for cc in range(2):
                cz = rcp.tile([128, KCH, SC], F32, name=f"c0_{l}_{cc}")
                nc.vector.memset(cz[:], 0.0)
                c_prev.append(cz)
            # per-step emission is grouped by engine stage (both chains
            # adjacent) so each engine's in-order queue never has an
            # instruction whose wait blocks the other chain's work.
            for t in range(T):
                gv = Gv8[t // 64][:, :, t % 64, :]  # [128, MCH, 8]
                ps = []
                for cc in range(2):
                    bs = slice(SC * cc, SC * (cc + 1))
                    p = rps.tile([128, MCH, SC], F32, tag=f"ps{cc}")
                    ps.append(p)
                    if t == 0:
                        nc.tensor.matmul(
                            out=p[:], lhsT=ident_bf[:], rhs=gv[:, :, bs],
                            start=True, stop=True, skip_group_check=True,
                        )
                    else:
                        # G-inject first: it has no h dependency, so it soaks
                        # up the stale PSUM-reuse wait and the first whh
                        # matmul carries only the live h wait.
                        nc.tensor.matmul(
                            out=p[:], lhsT=ident_bf[:], rhs=gv[:, :, bs],
                            start=True, stop=False, skip_group_check=True,
                        )
                        for m in range(MCH):
                            for kk in range(KCH):
                                nc.tensor.matmul(
                                    out=p[:, m, :],
                                    lhsT=whh[:, kk, 128 * m : 128 * (m + 1)],
                                    rhs=hvs[cc][:, kk, t - 1, :],
                                    start=False,
                                    stop=(kk == KCH - 1 and m == MCH - 1),
                                    skip_group_check=True,
                                )
                sg = []
                for cc in range(2):
                    s = rp.tile([128, MCH, SC], F32, tag=f"sg{cc}")
                    nc.scalar.activation(out=s[:], in_=ps[cc][:], func=AF.Sigmoid)
                    sg.append(s)
                t1s, t2s = [], []
                for cc in range(2):
                    # c = f*c_prev + i*(2*sg_g - 1); t2p/c fused via stt
                    t1 = rp.tile([128, KCH, SC], F32, tag=f"t1{cc}")
                    nc.vector.tensor_tensor(
                        out=t1[:], in0=sg[cc][:, 3:6, :], in1=c_prev[cc][:], op=ALU.mult
                    )
                    t1s.append(t1)
                for cc in range(2):
                    t2p = rp.tile([128, KCH, SC], F32, tag=f"t2{cc}")
                    nc.vector.scalar_tensor_tensor(
                        out=t2p[:], in0=sg[cc][:, 9:12, :], scalar=0.5,
                        in1=sg[cc][:, 0:3, :], op0=ALU.subtract, op1=ALU.mult,
                    )
                    t2s.append(t2p)
                c_new = []
                for cc in range(2):
                    cn = rp.tile([128, KCH, SC], F32, tag=f"c{cc}")
                    nc.vector.scalar_tensor_tensor(
                        out=cn[:], in0=t2s[cc][:], scalar=2.0, in1=t1s[cc][:],
                        op0=ALU.mult, op1=ALU.add,
                    )
                    c_new.append(cn)
                tc_t = []
                for cc in range(2):
                    tct = rp.tile([128, KCH, SC], F32, tag=f"tc{cc}")
                    nc.scalar.activation(out=tct[:], in_=c_new[cc][:], func=AF.Tanh)
                    tc_t.append(tct)
                for cc in range(2):
                    nc.vector.tensor_tensor(
                        out=hvs[cc][:, :, t, :], in0=sg[cc][:, 6:9, :], in1=tc_t[cc][:], op=ALU.mult
                    )
                rc = T - 1 - t
                for cc in range(2):
                    nc.vector.tensor_tensor(
                        out=hTr[cc][rc // QT][:, :, rc % QT, :],
                        in0=sg[cc][:, 6:9, :], in1=tc_t[cc][:], op=ALU.mult
                    )
                    c_prev[cc] = c_new[cc]
                if on_step is not None:
                    on_step(t)

    def exchange_chunk(l, j, hTr, xp_t):
        """Ship reversed-h chunk j (both chains) to the partner and gather
        the partner's chunk straight into xp_t[j] (cols = (chain, t~, b4),
        matching the own-h layout).  Gather outs must be contiguous —
        strided indirect-DMA outs write garbage (HW-verified)."""
        hq = QN // 2
        ct = ctrb[l][j].ap().rearrange("c p n -> p c n")
        nc.sync.dma_start(out=ct[:, :, 0:hq], in_=hTr[0][j][:])
        nc.sync.dma_start(out=ct[:, :, hq:QN], in_=hTr[1][j][:])
        nc.gpsimd.collective_compute(
            "AllGather",
            ALU.bypass,
            replica_groups=PAIRS,
            ins=[ctrb[l][j].ap()],
            outs=[hall[l][j].ap()],
        )
        rows = hall[l][j].ap().rearrange("r c p n -> (r c p) n")
        for cch in range(KCH):
            nc.gpsimd.indirect_dma_start(
                out=xp_t[j][:, cch, :],
                out_offset=None,
                in_=rows,
                in_offset=IndirectOffsetOnAxis(ap=gidx_sb[:, cch : cch + 1], axis=0),
            )

    def make_on_step(l, hTr, xp_t, work=None):
        work = list(work) if work else []

        def on_step(t):
            # up to one queued emission (embed ktile / G0 m-group) per step,
            # woven into the recurrence's engine slack
            if work and t % 2 == 0:
                work.pop(0)()
            # chunk j's reversed cols are complete after step T-1-QT*j
            if t >= QT - 1 and (t + 1) % QT == 0 and t != T - 1:
                j = (T - 1 - t) // QT
                exchange_chunk(l, j, hTr, xp_t)
            if t == T - 1:
                while work:
                    work.pop(0)()

        return on_step

    def exchange_last(l, hTr, xp_t):
        exchange_chunk(l, 0, hTr, xp_t)
        for cch in range(KCH):
            nc.tensor.ldweights(weights=xp_t[0][:, cch, 0:1])

    # ---- layer pipeline with scoped lifetimes (strict LIFO pools) ----
    with tc.tile_pool(name="phh", bufs=1) as phh:
        half = NT // 2
        hT0 = [phh.tile([128, KCH, half], F8E4, name=f"hT0{c}") for c in "ab"]
        hT1 = [phh.tile([128, KCH, half], F8E4, name=f"hT1{c}") for c in "ab"]
        hTr0 = [
            [phh.tile([128, KCH, QT, SCH], F8E4, name=f"hTr0{c}_{j}") for j in range(NCHUNK)]
            for c in "ab"
        ]
        hTr1 = [
            [phh.tile([128, KCH, QT, SCH], F8E4, name=f"hTr1{c}_{j}") for j in range(NCHUNK)]
            for c in "ab"
        ]
        xp_t = [phh.tile([128, KCH, QN], F8E4, name=f"xp{j}") for j in range(NCHUNK)]
        with tc.tile_pool(name="pg", bufs=1) as pgp:
            G_t = [pgp.tile([128, MCH, 512], BF16, name=f"G{nb}") for nb in range(NBG)]
            with tc.tile_pool(name="pr0", bufs=1) as pr0:
                whh0 = load_whh(0, pr0)
                with tc.tile_pool(name="pw0", bufs=1) as pw0:
                    wih0 = load_wih(0, pw0)
                    with tc.tile_pool(name="px", bufs=1) as px, tc.tile_pool(
                        name="s1", bufs=2
                    ) as s1, tc.tile_pool(
                        name="s1ps", bufs=1, space="PSUM"
                    ) as s1ps, tc.tile_pool(
                        name="g0ps", bufs=1, space="PSUM"
                    ) as g0ps:
                        xT_t = [
                            px.tile([128, DCH, 512], F8E4, name=f"xT{nb}")
                            for nb in range(NBG)
                        ]
                        embed_ktile = make_embed_ktile(s1, s1ps, xT_t)
                        # block 0 up front (the recurrence needs it at step 0);
                        # the rest weaves into the recurrence's engine slack
                        for k in range(4):
                            embed_ktile(k)
                        for m in range(MCH):
                            g0_mgroup(G_t, xT_t, wih0, g0ps, 0, m)
                        work = []
                        for nb in range(1, NBG):
                            for k in range(4 * nb, 4 * nb + 4):
                                work.append(lambda k=k: embed_ktile(k))
                            for m in range(0, MCH, 3):
                                work.append(
                                    lambda nb=nb, m=m: [
                                        g0_mgroup(G_t, xT_t, wih0, g0ps, nb, mm)
                                        for mm in range(m, m + 3)
                                    ]
                                )
                        recurrence(
                            0, G_t, hT0, hTr0, whh0,
                            on_step=make_on_step(0, hTr0, xp_t, work=work),
                        )
            exchange_last(0, hTr0, xp_t)
            with tc.tile_pool(name="pw1", bufs=1) as pw1, tc.tile_pool(
                name="g1ps", bufs=2, space="PSUM"
            ) as g1ps:
                wih1 = load_wih(1, pw1)
                nc.tensor.ldweights(weights=wih1[:, 0, 0:1])
                for nb in (1, 2, 3, 0):
                    for m in range(MCH):
                        for cc in range(2):
                            g1_mgroup(G_t, hT0, xp_t, wih1, g1ps, nb, m, cc)
            # label-only score terms (one-hot, transition pairs, start/end)
            # depend on nothing but labels: weave them into L1's engine slack
            labf_sb = phh.tile([1, NT], F32, name="labf_sb")
            ohT = phh.tile([C, NT], F32, name="ohT")
            m1T = phh.tile([C, NT], F32, name="m1T")
            s8T = phh.tile([C, 2, GB], F32, name="s8T")
            scR = phh.tile([C, 3], F32, name="scR")  # pd, st, en sums

            def sc_bcast(nb, scps):
                blk = slice(512 * nb, 512 * (nb + 1))
                bps = scps.tile([C, 512], F32, tag="scps")
                nc.tensor.matmul(
                    out=bps[:], lhsT=ones1C[:], rhs=labf_sb[:, blk],
                    start=True, stop=True,
                )
                nc.vector.tensor_copy(out=m1T[:, blk], in_=bps[:])

            def sc_oh():
                nc.vector.tensor_scalar(
                    out=ohT[:], in0=m1T[:], scalar1=iota_sb[:], scalar2=None,
                    op0=ALU.is_equal,
                )

            def sc_m1(nb, scps):
                lo = 512 * nb
                hi = min(512 * (nb + 1), NT - 8)
                mps = scps.tile([C, 512], F32, tag="scps")
                nc.tensor.matmul(
                    out=mps[:, : hi - lo], lhsT=transT_sb[:],
                    rhs=ohT[:, lo + 8 : hi + 8], start=True, stop=True,
                )
                nc.vector.tensor_copy(out=m1T[:, lo:hi], in_=mps[:, : hi - lo])

            def sc_fin():
                nc.vector.tensor_tensor(
                    out=m1T[:, : NT - 8], in0=ohT[:, : NT - 8],
                    in1=m1T[:, : NT - 8], op=ALU.mult,
                )
                nc.vector.reduce_sum(
                    out=scR[:, 0:1], in_=m1T[:, : NT - 8], axis=mybir.AxisListType.X
                )
                nc.vector.tensor_scalar_mul(
                    out=s8T[:, 0, :], in0=ohT[:, 0:GB], scalar1=stv_sb[:]
                )
                nc.vector.reduce_sum(
                    out=scR[:, 1:2], in_=s8T[:, 0, :], axis=mybir.AxisListType.X
                )
                nc.vector.tensor_scalar_mul(
                    out=s8T[:, 1, :], in0=ohT[:, NT - GB : NT], scalar1=env_sb[:]
                )
                nc.vector.reduce_sum(
                    out=scR[:, 2:3], in_=s8T[:, 1, :], axis=mybir.AxisListType.X
                )

            with tc.tile_pool(name="scps", bufs=1, space="PSUM") as scps:
                nc.sync.dma_start(out=labf_sb[:], in_=ins["labf"])
                swork = (
                    [lambda nb=nb: sc_bcast(nb, scps) for nb in range(4)]
                    + [sc_oh]
                    + [lambda nb=nb: sc_m1(nb, scps) for nb in range(4)]
                    + [sc_fin]
                )
                with tc.tile_pool(name="pr1", bufs=1) as pr1:
                    whh1 = load_whh(1, pr1)
                    recurrence(
                        1, G_t, hT1, hTr1, whh1,
                        on_step=make_on_step(1, hTr1, xp_t, work=swork),
                    )
        exchange_last(1, hTr1, xp_t)

        # ---- emissions: em^T [C, NT] = fc @ concat(h2_own, h2_partner) ----
        crf_cm = tc.tile_pool(name="crf", bufs=1)
        crf = crf_cm.__enter__()
        pe_touch_f32(cpack_sb[:, 0:1])
        tileA = crf.tile([C, NT], F32, name="tileA")  # emT
        tileB = crf.tile([C, NT], F32, name="tileB")  # Q
        tileC = crf.tile([C, NT], F32, name="tileC")  # gem
        emT = tileA
        with tc.tile_pool(name="emps", bufs=2, space="PSUM") as emps:
            nc.tensor.ldweights(weights=fcT_sb[:, 0, 0:1])
            emv8 = emT[:].rearrange("c (t b) -> c t b", b=GB)
            for nb in (1, 2, 3, 0):
                for cc in range(2):
                    ps = emps.tile([C, 256], F32, tag=f"emps{cc}")
                    # no DoubleRow: fcT's k-pair stride (C=14B) violates the
                    # dual-fp8 LDWEIGHTS stride%16 ISA restriction
                    ob = slice(256 * nb, 256 * (nb + 1))
                    pb = slice(256 * cc, 256 * (cc + 1))
                    for kk in range(DCH):
                        rhs = (
                            hT1[cc][:, kk, ob]
                            if kk < KCH
                            else xp_t[nb][:, kk - KCH, pb]
                        )
                        nc.tensor.matmul(
                            out=ps[:],
                            lhsT=fcT_sb[:, kk, :],
                            rhs=rhs,
                            start=(kk == 0),
                            stop=(kk == DCH - 1),
                        )
                    nc.vector.tensor_scalar_add(
                        out=emv8[:, 64 * nb : 64 * (nb + 1), 4 * cc : 4 * cc + 4],
                        in0=ps[:].rearrange("c (t b) -> c t b", b=SCH),
                        scalar1=fcb_sb[:],
                    )
        if DEBUG_OUTS:
            nc.sync.dma_start(out=dbg["dbg_em"], in_=emT[:])

        # ---- CRF ----
        with tc.tile_pool(name="crfw", bufs=4) as cw, tc.tile_pool(
            name="crfps", bufs=1, space="PSUM"
        ) as cps:
            Q = tileB
            nc.scalar.activation(out=Q[:], in_=emT[:], func=AF.Exp)
            dve_touch(Q[0:1, 0:1])
            Qv = Q[:].rearrange("c (t b) -> c t b", b=GB)

            # gold emissions total (pd/st/en were precomputed during L1)
            gem = tileC
            nc.vector.tensor_tensor(out=gem[:], in0=emT[:], in1=ohT[:], op=ALU.mult)
            gem_r = cw.tile([C, 1], F32, tag="gred")
            nc.vector.reduce_sum(out=gem_r[:], in_=gem[:], axis=mybir.AxisListType.X)

            score_ps = cps.tile([1, 8], F32, tag="scoreps")
            for i, r in enumerate((gem_r[:], scR[:, 0:1], scR[:, 1:2], scR[:, 2:3])):
                nc.tensor.matmul(
                    out=score_ps[:1, :1],
                    lhsT=onesC1[:],
                    rhs=r,
                    start=(i == 0),
                    stop=(i == 3),
                    skip_group_check=True,
                )
            score_sb = cw.tile([1, 1], F32, tag="scoresb")
            nc.vector.tensor_copy(out=score_sb[:], in_=score_ps[:1, :1])

            # ---- blocked forward chain in exp domain, E pre-scaled by
            # 1/ESCALE so no renorms are needed (f32 range absorbs the
            # drift; the log-correction is added at the end).
            # Block 0 runs the alpha chain over t in [0, BT); blocks k=1..3
            # run transfer-matrix chains Yk = M_k^T = E'D_lo ... E'D_hi
            # (built high-t to low-t), all NBLK chains concurrently.
            expTTp_bf = cw.tile([C, C], BF16, tag="ettbf")
            nc.vector.tensor_copy(out=expTTp_bf[:], in_=expTTp_f32)
            v_prev = cw.tile([C, GB], F32, tag="v")
            nc.vector.tensor_scalar_mul(out=v_prev[:], in0=Qv[:, 0, :], scalar1=expst_sb[:])
            eye_bc = eye_sb.unsqueeze(1).broadcast_to([C, GB, C])
            yps_prev = [None] * NBLK
            for s in range(BT):
                if s > 0:
                    vps = cps.tile([C, GB], F32, tag="vps")
                    nc.tensor.matmul(out=vps[:], lhsT=E_sb[:], rhs=v_prev[:], start=True, stop=True)
                    v_new = cw.tile([C, GB], F32, tag="v")
                    nc.vector.tensor_tensor(out=v_new[:], in0=vps[:], in1=Qv[:, s, :], op=ALU.mult)
                    v_prev = v_new
                for k in range(1, NBLK):
                    t = BT * (k + 1) - 1 - s
                    qb = Qv[:, t, :].unsqueeze(2).broadcast_to([C, GB, C])
                    w = cw.tile([C, GB, C], BF16, tag=f"w{k}")
                    if s == 0:
                        nc.vector.tensor_tensor(out=w[:], in0=eye_bc, in1=qb, op=ALU.mult)
                    else:
                        nc.vector.tensor_tensor(
                            out=w[:],
                            in0=yps_prev[k][:].rearrange("c (b j) -> c b j", b=GB),
                            in1=qb,
                            op=ALU.mult,
                        )
                    yp = cps.tile([C, GB * C], F32, tag=f"yps{k}")
                    nc.tensor.matmul(out=yp[:], lhsT=expTTp_bf[:], rhs=w[:], start=True, stop=True)
                    yps_prev[k] = yp
            yfin = []
            for k in range(1, NBLK):
                yf = cw.tile([C, GB, C], BF16, tag=f"yf{k}")
                nc.vector.tensor_copy(
                    out=yf[:], in_=yps_prev[k][:].rearrange("c (b j) -> c b j", b=GB)
                )
                yfin.append(yf)
            # fold block results into the alpha vector
            for k in range(1, NBLK):
                vb = cw.tile([C, GB], BF16, tag="vb")
                nc.vector.tensor_copy(out=vb[:], in_=v_prev[:])
                aps = cps.tile([C, GB], F32, tag="vps")
                for b in range(GB):
                    nc.tensor.matmul(
                        out=aps[:, b : b + 1],
                        lhsT=yfin[k - 1][:, b, :],
                        rhs=vb[:, b : b + 1],
                        start=(b == 0),
                        stop=(b == GB - 1),
                        skip_group_check=True,
                    )
                v_new = cw.tile([C, GB], F32, tag="v")
                nc.vector.tensor_copy(out=v_new[:], in_=aps[:])
                v_prev = v_new
            vend = cw.tile([C, GB], F32, tag="vend")
            nc.vector.tensor_scalar_mul(out=vend[:], in0=v_prev[:], scalar1=expen_sb[:])
            zps = cps.tile([1, GB], F32, tag="cps1")
            nc.tensor.matmul(out=zps[:], lhsT=onesC1[:], rhs=vend[:], start=True, stop=True)
            lnz = cw.tile([1, GB], F32, tag="lnz")
            nc.scalar.activation(out=lnz[:], in_=zps[:], func=AF.Ln)
            logz = cw.tile([1, GB], F32, tag="logz")
            nc.vector.tensor_scalar(
                out=logz[:], in0=lnz[:], scalar1=float((T - 1) * np.log(ESCALE)),
                scalar2=None, op0=ALU.add,
            )
            lz_tot = cw.tile([1, 1], F32, tag="lztot")
            nc.vector.reduce_sum(out=lz_tot[:], in_=logz[:], axis=mybir.AxisListType.X)
            loss_sb = cw.tile([1, 1], F32, tag="loss_sb")
            nc.vector.tensor_tensor(out=loss_sb[:], in0=lz_tot[:], in1=score_sb[:], op=ALU.subtract)
            nc.sync.dma_start(out=loss_out, in_=loss_sb[:])
            if DEBUG_OUTS:
                dsc = cw.tile([1, 2], F32, tag="dsc")
                nc.vector.tensor_copy(out=dsc[:, 0:1], in_=lz_tot[:])
                nc.vector.tensor_copy(out=dsc[:, 1:2], in_=score_sb[:])
                nc.sync.dma_start(out=dbg["dbg_sc"], in_=dsc[:])
        crf_cm.__exit__(None, None, None)

    est.close()


# ---------------------------------------------------------------------------
# host side
# ---------------------------------------------------------------------------

def make_in_maps(inputs):
    ids = np.asarray(inputs["input_ids"]).astype(np.int64)
    labels = np.asarray(inputs["labels"]).astype(np.int64)
    word_emb = _f32(inputs["word_emb"])
    pos_emb = _f32(inputs["pos_emb"])
    type_emb = _f32(inputs["type_emb"])
    ln_g = _f32(inputs["ln_g"])
    ln_b = _f32(inputs["ln_b"])
    w_ih = _f32(inputs["w_ih"])
    w_hh = _f32(inputs["w_hh"])
    b_ih = _f32(inputs["b_ih"])
    b_hh = _f32(inputs["b_hh"])
    fc_w = _f32(inputs["fc_w"])
    fc_b = _f32(inputs["fc_b"])
    crf_start = _f32(inputs["crf_start"])
    crf_end = _f32(inputs["crf_end"])
    crf_trans = _f32(inputs["crf_trans"])

    posty0 = pos_emb[:T] + type_emb[0][None, :]
    word_emb_bf = _bf(word_emb)

    in_maps = []
    for core in range(NCORES):
        g, d = core // 2, core % 2
        sl = slice(GB * g, GB * (g + 1))
        ids_loc = ids[sl]
        lab_loc = labels[sl]
        posty = posty0
        if d == 1:
            ids_loc = ids_loc[:, ::-1]
            lab_loc = lab_loc[:, ::-1]
            posty = posty0[::-1]

        # layer-0 weights with LN affine folded in (gate-permuted, g x2)
        w0 = _perm_gates(w_ih[0, d] * ln_g[None, :])
        bias0 = _perm_gates((b_ih[0, d] + b_hh[0, d] + w_ih[0, d] @ ln_b)[:, None])[:, 0]
        # layer-1 weights, columns permuted to local [own, partner] order
        w1 = w_ih[1, d]
        if d == 1:
            w1 = np.concatenate([w1[:, HD:], w1[:, :HD]], axis=1)
        w1 = _perm_gates(w1)
        bias1 = _perm_gates((b_ih[1, d] + b_hh[1, d])[:, None])[:, 0]
        fcp = fc_w if d == 0 else np.concatenate([fc_w[:, HD:], fc_w[:, :HD]], axis=1)

        trans_eff = crf_trans if d == 0 else crf_trans.T
        start_eff = crf_start if d == 0 else crf_end
        end_eff = crf_end if d == 0 else crf_start

        pr = 1 - d
        gidx = np.empty((128, KCH), np.int32)
        for cch in range(KCH):
            gidx[:, cch] = pr * (KCH * 128) + cch * 128 + np.arange(128)

        cpack = np.zeros((C, CPW), np.float32)
        cpack[:, 0:C] = np.exp(trans_eff) / ESCALE
        cpack[:, C : 2 * C] = trans_eff.T
        cpack[:, 28] = np.exp(start_eff)
        cpack[:, 29] = np.exp(end_eff)
        cpack[:, 30] = start_eff
        cpack[:, 31] = end_eff
        cpack[:, 32] = np.arange(C, dtype=np.float32)
        cpack[:, 33] = fc_b
        cpack[:, 34 : 34 + C] = (np.exp(trans_eff) / ESCALE).T
        cpack[:, 34 + C : 34 + 2 * C] = np.eye(C, dtype=np.float32)

        b01 = np.concatenate(
            [bias0.reshape(MCH, 128).T, bias1.reshape(MCH, 128).T], axis=1
        )

        # t-major token order: token n = (t, b)
        ids_tm = np.ascontiguousarray(ids_loc.T).reshape(NT, 1)
        lab_tm = np.ascontiguousarray(lab_loc.T).reshape(1, NT)
        posty_rep = np.repeat(np.asarray(posty, np.float32), GB, axis=0)
        in_maps.append(
            dict(
                ids32=np.ascontiguousarray(ids_tm.astype(np.int32)),
                labf=np.ascontiguousarray(lab_tm.astype(np.float32)),
                word_emb=word_emb_bf,
                posty=_bf(posty_rep),
                wih0T=_f8(w0.T),
                wih1T=_f8(w1.T),
                whh0T=_bf(_perm_gates(w_hh[0, d]).T),
                whh1T=_bf(_perm_gates(w_hh[1, d]).T),
                b01=np.ascontiguousarray(b01),
                fcT=_f8(fcp.T),
                cpack=cpack,
                gidx=gidx,
            )
        )
    return in_maps


_PROGRAM = None
_COST_MODEL_NS = None


def _get_program():
    global _PROGRAM, _COST_MODEL_NS
    if _PROGRAM is None:
        _PROGRAM = build_program()
        try:
            from concourse.timeline_sim import TimelineSim

            _COST_MODEL_NS = int(TimelineSim(_PROGRAM, trace=False, no_exec=True).simulate())
        except Exception:
            _COST_MODEL_NS = None
    return _PROGRAM


def run(inputs, trace=False):
    nc = _get_program()
    in_maps = make_in_maps(inputs)
    res = run_bass_kernel_spmd(nc, in_maps, core_ids=list(range(NCORES)), trace=trace)
    total = np.float64(0.0)
    for g in range(4):
        total += np.float64(res.results[2 * g]["loss"][0, 0])
    return np.asarray(total, dtype=np.float32), res


def kernel(**inputs):
    out, _ = run(inputs, trace=False)
    return out



# revision 67
# speedup vs baseline: 1.0278x; 1.0221x over previous
"""BiLSTM-CRF sequence-tagging loss on 8 Trainium2 NeuronCores.

Sharding: 8 cores = 4 batch-groups x 2 LSTM directions.
  core 2g+d handles sequences [8g, 8g+8) ; d=0 forward, d=1 backward.
Backward cores receive time-reversed inputs (ids/pos/labels), so one SPMD
program runs on all cores; their CRF uses transposed transitions with
start/end swapped (same loss by path reversal), and their layer-2/emission
weights are column-permuted so the local [own_h, partner_h] concat order is
uniform.  The h-streams are exchanged pairwise via AllGather through DRAM;
the partner slot is fetched with an indirect-DMA row gather whose indices
are per-core input data (keeps the program core-uniform).
"""

import os
import sys

import numpy as np

for _p in ("/opt/trn_rl_repo", "/root/.axon_site/_ro/trn_rl_repo"):
    if os.path.isdir(_p) and _p not in sys.path:
        sys.path.insert(0, _p)

import ml_dtypes  # noqa: E402

import concourse.bass as bass  # noqa: E402
import concourse.bacc as bacc  # noqa: E402
import concourse.tile as tile  # noqa: E402
from concourse import mybir  # noqa: E402
from concourse.bass import IndirectOffsetOnAxis  # noqa: E402
from concourse.bass_utils import run_bass_kernel_spmd  # noqa: E402
from concourse.masks import make_identity  # noqa: E402

F32 = mybir.dt.float32
BF16 = mybir.dt.bfloat16
F8E4 = mybir.dt.float8e4
I32 = mybir.dt.int32
AF = mybir.ActivationFunctionType
ALU = mybir.AluOpType

# problem shapes (hardcoded per contract)
B, T, V, D, C, HD = 32, 256, 30522, 768, 14, 384
L = 2
NCORES = 8
GB = 8            # sequences per core group
NT = GB * T       # tokens per core = 2048
NTILE = NT // 128  # 16
MCH = 12          # gate chunks of 128 (4*HD/128)
KCH = 3           # hidden chunks (HD/128)
DCH = 6           # input-dim chunks (D/128)
LN_EPS = 1e-12
PAIRS = [[0, 1], [2, 3], [4, 5], [6, 7]]
ESCALE = 16.0     # folded per-step scaling of exp(trans); no renorms needed
NBLK = 4          # CRF scan blocks (1 alpha chain + NBLK-1 matrix chains)
BT = T // NBLK    # 64 steps per block
CPW = 34 + 2 * 14  # cpack width
NCHUNK = 4        # exchange chunks per layer
QT = T // NCHUNK  # 64 timesteps per chunk
QN = GB * QT      # 512 columns per chunk
SCH = GB // 2     # sequences per recurrence chain
NBG = NT // 512   # 512-token (64-step) blocks

DEBUG_OUTS = False


def _bf(x):
    return np.ascontiguousarray(np.asarray(x, dtype=np.float32)).astype(ml_dtypes.bfloat16)


def _f32(x):
    return np.ascontiguousarray(np.asarray(x, dtype=np.float32))


def _f8(x):
    return np.ascontiguousarray(np.asarray(x, dtype=np.float32)).astype(
        ml_dtypes.float8_e4m3
    )


def _perm_gates(w):
    """torch gate order i,f,g,o -> device order i,f,o,g with the g block
    scaled by 2 (tanh(x) = 2*sigmoid(2x)-1)."""
    H = HD
    return np.concatenate([w[0:H], w[H:2 * H], w[3 * H:4 * H], 2.0 * w[2 * H:3 * H]], axis=0)


# ---------------------------------------------------------------------------
# device program
# ---------------------------------------------------------------------------

def build_program():
    nc = bacc.Bacc("TRN2", target_bir_lowering=False, debug=False, num_devices=NCORES)

    def din(name, shape, dt):
        return nc.dram_tensor(name, shape, dt, kind="ExternalInput").ap()

    ins = dict(
        ids32=din("ids32", [NT, 1], I32),
        labf=din("labf", [1, NT], F32),
        word_emb=din("word_emb", [V, D], BF16),
        posty=din("posty", [NT, D], BF16),
        wih0T=din("wih0T", [D, 4 * HD], F8E4),
        wih1T=din("wih1T", [D, 4 * HD], F8E4),
        whh0T=din("whh0T", [HD, 4 * HD], F8E4),
        whh1T=din("whh1T", [HD, 4 * HD], F8E4),
        b01=din("b01", [128, 2 * MCH], F32),
        fcT=din("fcT", [D, C], F8E4),
        cpack=din("cpack", [C, CPW], F32),
        gidx=din("gidx", [128, KCH], I32),
    )

    loss_out = nc.dram_tensor("loss", [1, 1], F32, kind="ExternalOutput").ap()
    dbg = {}
    if DEBUG_OUTS:
        dbg["dbg_xt"] = nc.dram_tensor("dbg_xt", [128, DCH, NT], BF16, kind="ExternalOutput").ap()
        dbg["dbg_g"] = nc.dram_tensor("dbg_g", [128, MCH, NT], BF16, kind="ExternalOutput").ap()
        dbg["dbg_h1"] = nc.dram_tensor("dbg_h1", [128, KCH, NT], BF16, kind="ExternalOutput").ap()
        dbg["dbg_h2"] = nc.dram_tensor("dbg_h2", [128, KCH, NT], BF16, kind="ExternalOutput").ap()
        dbg["dbg_em"] = nc.dram_tensor("dbg_em", [C, NT], F32, kind="ExternalOutput").ap()
        dbg["dbg_sc"] = nc.dram_tensor("dbg_sc", [1, 2], F32, kind="ExternalOutput").ap()

    # internal DRAM for pairwise exchange (fp8, 4 time-chunks per layer so
    # collectives overlap the recurrence)
    ctrb = [
        [nc.dram_tensor(f"ctrb{l}_{j}", [KCH, 128, QN], F8E4) for j in range(NCHUNK)]
        for l in range(L)
    ]
    hall = [
        [nc.dram_tensor(f"hall{l}_{j}", [2, KCH, 128, QN], F8E4) for j in range(NCHUNK)]
        for l in range(L)
    ]

    with tile.TileContext(nc) as tc:
        _build_body(tc, ins, loss_out, dbg, ctrb, hall)

    nc.compile()
    return nc


def _build_body(tc, ins, loss_out, dbg, ctrb, hall):
    nc = tc.nc
    from contextlib import ExitStack

    est = ExitStack()
    pers = est.enter_context(tc.tile_pool(name="pers", bufs=1))

    # ---- persistent SBUF state (small constants only) ----
    def load_wih(l, pool):
        wt = pool.tile([128, DCH, 4 * HD], F8E4, name=f"wih{l}")
        src = ins["wih0T"] if l == 0 else ins["wih1T"]
        nc.sync.dma_start(out=wt[:], in_=src.rearrange("(k p) m -> p k m", p=128))
        return wt

    def load_whh(l, pool):
        ht = pool.tile([128, KCH, 4 * HD], F8E4, name=f"whh{l}")
        src = ins["whh0T"] if l == 0 else ins["whh1T"]
        nc.sync.dma_start(out=ht[:], in_=src.rearrange("(k p) m -> p k m", p=128))
        return ht

    # scratch + absorbers: this toolchain allows only ONE sem wait per
    # instruction, so every junction of two producers gets a tiny absorber op
    # that folds one producer into the consuming engine's clock first.
    scr_dve = pers.tile([1, 4], F32, name="scr_dve")
    scr_gp = pers.tile([1, 4], I32, name="scr_gp")
    pabs = est.enter_context(tc.tile_pool(name="pabs", bufs=1, space="PSUM"))
    pscr = pabs.tile([1, 8], F32, name="pscr")

    def dve_touch(ap):
        nc.vector.tensor_copy(out=scr_dve[:, 0:1], in_=ap)

    def pe_touch_f32(ap_col):
        nc.tensor.matmul(out=pscr[:1, :1], lhsT=ap_col, rhs=ap_col, start=True, stop=True)

    b_sb = pers.tile([128, 2 * MCH], F32, name="b_sb")
    nc.sync.dma_start(out=b_sb[:], in_=ins["b01"])
    dve_touch(b_sb[0:1, 0:1])

    fcT_sb = pers.tile([128, DCH, C], F8E4, name="fcT")
    nc.sync.dma_start(out=fcT_sb[:], in_=ins["fcT"].rearrange("(k p) m -> p k m", p=128))

    cpack_sb = pers.tile([C, CPW], F32, name="cpack_sb")
    nc.sync.dma_start(out=cpack_sb[:], in_=ins["cpack"])
    dve_touch(cpack_sb[0:1, 0:1])
    E_sb = cpack_sb[:, 0:C]  # exp(trans)/ESCALE
    transT_sb = cpack_sb[:, C : 2 * C]
    expst_sb = cpack_sb[:, 28:29]
    expen_sb = cpack_sb[:, 29:30]
    stv_sb = cpack_sb[:, 30:31]
    env_sb = cpack_sb[:, 31:32]
    iota_sb = cpack_sb[:, 32:33]
    fcb_sb = cpack_sb[:, 33:34]
    expTTp_f32 = cpack_sb[:, 34 : 34 + C]  # exp(trans.T)/ESCALE
    eye_sb = cpack_sb[:, 34 + C : 34 + 2 * C]  # identity

    gidx_sb = pers.tile([128, KCH], I32, name="gidx_sb")
    nc.sync.dma_start(out=gidx_sb[:], in_=ins["gidx"])
    nc.gpsimd.tensor_copy(out=scr_gp[:, 0:1], in_=gidx_sb[0:1, 0:1])

    ids_sb = pers.tile([128, NTILE], I32, name="ids_sb")
    nc.sync.dma_start(out=ids_sb[:], in_=ins["ids32"].rearrange("(k p) o -> p (k o)", p=128))

    ident = pers.tile([128, 128], F32, name="ident")
    make_identity(nc, ident[:])
    ident_bf = pers.tile([128, 128], BF16, name="ident_bf")
    nc.vector.tensor_copy(out=ident_bf[:], in_=ident[:])
    pe_touch_f32(ident[:, 0:1])
    eps_sb = pers.tile([128, 1], F32, name="eps_sb")
    nc.vector.memset(eps_sb[:], LN_EPS)
    ones1C = pers.tile([1, C], F32, name="ones1C")
    nc.vector.memset(ones1C[:], 1.0)
    onesC1 = pers.tile([C, 1], F32, name="onesC1")
    nc.vector.memset(onesC1[:], 1.0)

    # ---- helpers ----
    def make_embed_ktile(s1, s1ps, xT_t):
        """Returns emit(k): embeds token k-tile k (128 t-major tokens) into
        xT_t[k//4]. Called lazily so ktiles can be woven into the layer-0
        recurrence's engine slack."""

        def emit(k):
            posty_sb = s1.tile([128, D], BF16, tag="posty")
            nc.sync.dma_start(
                out=posty_sb[:], in_=ins["posty"][128 * k : 128 * (k + 1), :]
            )
            emb = s1.tile([128, D], BF16, tag="emb")
            nc.gpsimd.indirect_dma_start(
                out=emb[:],
                out_offset=None,
                in_=ins["word_emb"],
                in_offset=IndirectOffsetOnAxis(ap=ids_sb[:, k : k + 1], axis=0),
            )
            emb2 = s1.tile([128, D], BF16, tag="emb2")
            nc.vector.tensor_add(out=emb2[:], in0=emb[:], in1=posty_sb[:])
            stats = s1.tile([128, 3, 6], F32, tag="stats")
            embv = emb2[:].rearrange("p (s q) -> p s q", s=3)
            for sg in range(3):
                nc.vector.bn_stats(out=stats[:, sg, :], in_=embv[:, sg, :])
            mv = s1.tile([128, 2], F32, tag="mv")
            nc.vector.bn_aggr(out=mv[:], in_=stats[:])
            std = s1.tile([128, 1], F32, tag="std")
            nc.scalar.activation(out=std[:], in_=mv[:, 1:2], func=AF.Sqrt, bias=eps_sb[:])
            rstd = s1.tile([128, 1], F32, tag="rstd")
            nc.vector.reciprocal(out=rstd[:], in_=std[:])
            xln = s1.tile([128, D], BF16, tag="xln")
            nc.vector.tensor_scalar(
                out=xln[:],
                in0=emb2[:],
                scalar1=mv[:, 0:1],
                scalar2=rstd[:],
                op0=ALU.subtract,
                op1=ALU.mult,
            )
            for j in range(DCH):
                tp = s1ps.tile([128, 128], BF16, tag="tp")
                nc.tensor.transpose(
                    out=tp[:], in_=xln[:, 128 * j : 128 * (j + 1)], identity=ident_bf[:]
                )
                # spread PSUM->SBUF copies across engines; DVE is the
                # embed bottleneck
                dst = xT_t[k // 4][:, j, 128 * (k % 4) : 128 * (k % 4 + 1)]
                if j % 2 == 0:
                    nc.vector.tensor_copy(out=dst, in_=tp[:])
                else:
                    nc.scalar.copy(out=dst, in_=tp[:])

        return emit

    DR = mybir.MatmulPerfMode.DoubleRow

    def g0_mgroup(G_t, xT_t, wih, gps, nb, m):
        """One gate-chunk m of G0 block nb: 3 fp8 DoubleRow matmuls + bias."""
        ms = slice(128 * m, 128 * (m + 1))
        ps = gps.tile([128, 512], F32, tag="gps")
        for kp in range(DCH // 2):
            nc.tensor.matmul(
                out=ps[:],
                lhsT=wih[:, 2 * kp : 2 * kp + 2, ms],
                rhs=xT_t[nb][:, 2 * kp : 2 * kp + 2, :],
                start=(kp == 0),
                stop=(kp == DCH // 2 - 1),
                perf_mode=DR,
            )
        nc.vector.tensor_scalar_add(
            out=G_t[nb][:, m, :], in0=ps[:], scalar1=b_sb[:, m : m + 1]
        )

    def cat_segs(wT, hT, xp_t, nb, cc, ms):
        """Segments for concat(h_own[3 chunks], xp[3 chunks]) for chain cc of
        block nb: pairs (0,1)/(4,5) ride DoubleRow, 2/3 go single (they
        straddle the own/partner tile boundary)."""
        ob = slice(256 * nb, 256 * (nb + 1))
        pb = slice(256 * cc, 256 * (cc + 1))
        return [
            (wT[:, 0:2, ms], hT[cc][:, 0:2, ob], DR),
            (wT[:, 2, ms], hT[cc][:, 2, ob], None),
            (wT[:, 3, ms], xp_t[nb][:, 0, pb], None),
            (wT[:, 4:6, ms], xp_t[nb][:, 1:3, pb], DR),
        ]

    def g1_mgroup(G_t, hT, xp_t, wih, gps, nb, m, cc):
        """One (gate-chunk m, chain cc) piece of G1 block nb."""
        ms = slice(128 * m, 128 * (m + 1))
        ps = gps.tile([128, 256], F32, tag=f"gps{cc}")
        segs = cat_segs(wih, hT, xp_t, nb, cc, ms)
        for si, (lh, rhs, perf) in enumerate(segs):
            nc.tensor.matmul(
                out=ps[:], lhsT=lh, rhs=rhs,
                start=(si == 0), stop=(si == len(segs) - 1), perf_mode=perf,
            )
        gv8 = G_t[nb][:, m, :].rearrange("p (t b) -> p t b", b=GB)
        nc.vector.tensor_scalar_add(
            out=gv8[:, :, 4 * cc : 4 * cc + 4],
            in0=ps[:].rearrange("p (t b) -> p t b", b=SCH),
            scalar1=b_sb[:, MCH + m : MCH + m + 1],
        )

    def recurrence(l, G_t, hT, hTr, whh, on_step=None):
        """LSTM scan over time (t-major token layout); writes hT (pair of
        per-chain tiles, cols (t, b4)) and hTr (pair of per-chain lists of
        per-chunk fp8 tiles, time-reversed).

        Gates in device order (i,f,o,g) with g pre-scaled x2: one Sigmoid
        covers every gate (tanh(x) = 2*sigmoid(2x)-1).  G_t is injected into
        the gate PSUM by an identity matmul; one accumulation group per tile
        (start=True zeroes the whole 2KB bank region).

        The two chains use SEPARATE h tiles: the tile framework tracks deps
        at tile granularity, so a shared tile would make chain 0's matmul
        reads wait on chain 1's h writes and serialize the stagger."""
        Gv8 = [g[:].rearrange("p m (t b) -> p m t b", b=GB) for g in G_t]
        hvs = [h[:].rearrange("p c (t b) -> p c t b", b=SCH) for h in hT]
        with tc.tile_pool(name=f"r{l}", bufs=8) as rp, tc.tile_pool(
            name=f"r{l}c", bufs=1) as rcp, tc.tile_pool(
            name=f"r{l}ps", bufs=2, space="PSUM"
        ) as rps:
            nc.tensor.ldweights(weights=whh[:, 0, 0:1])
            SC = SCH  # two sliding chains of 4 sequences
            c_prev = []
            for cc in range(2):
                cz = rcp.tile([128, KCH, SC], F32, name=f"c0_{l}_{cc}")
                nc.vector.memset(cz[:], 0.0)
                c_prev.append(cz)
            # per-step emission is grouped by engine stage (both chains
            # adjacent) so each engine's in-order queue never has an
            # instruction whose wait blocks the other chain's work.
            for t in range(T):
                gv = Gv8[t // 64][:, :, t % 64, :]  # [128, MCH, 8]
                ps = []
                for cc in range(2):
                    bs = slice(SC * cc, SC * (cc + 1))
                    p = rps.tile([128, MCH, SC], F32, tag=f"ps{cc}")
                    ps.append(p)
                    if t == 0:
                        nc.tensor.matmul(
                            out=p[:], lhsT=ident_bf[:], rhs=gv[:, :, bs],
                            start=True, stop=True, skip_group_check=True,
                        )
                    else:
                        # G-inject first: it has no h dependency, so it soaks
                        # up the stale PSUM-reuse wait and the first whh
                        # matmul carries only the live h wait.
                        nc.tensor.matmul(
                            out=p[:], lhsT=ident_bf[:], rhs=gv[:, :, bs],
                            start=True, stop=False, skip_group_check=True,
                        )
                        for m in range(MCH):
                            ms = slice(128 * m, 128 * (m + 1))
                            nc.tensor.matmul(
                                out=p[:, m, :],
                                lhsT=whh[:, 0:2, ms],
                                rhs=hvs[cc][:, 0:2, t - 1, :],
                                start=False,
                                stop=False,
                                perf_mode=DR,
                                skip_group_check=True,
                            )
                            nc.tensor.matmul(
                                out=p[:, m, :],
                                lhsT=whh[:, 2, ms],
                                rhs=hvs[cc][:, 2, t - 1, :],
                                start=False,
                                stop=(m == MCH - 1),
                                skip_group_check=True,
                            )
                sg = []
                for cc in range(2):
                    s = rp.tile([128, MCH, SC], F32, tag=f"sg{cc}")
                    nc.scalar.activation(out=s[:], in_=ps[cc][:], func=AF.Sigmoid)
                    sg.append(s)
                t1s, t2s = [], []
                for cc in range(2):
                    # c = f*c_prev + i*(2*sg_g - 1); t2p/c fused via stt
                    t1 = rp.tile([128, KCH, SC], F32, tag=f"t1{cc}")
                    nc.vector.tensor_tensor(
                        out=t1[:], in0=sg[cc][:, 3:6, :], in1=c_prev[cc][:], op=ALU.mult
                    )
                    t1s.append(t1)
                for cc in range(2):
                    t2p = rp.tile([128, KCH, SC], F32, tag=f"t2{cc}")
                    nc.vector.scalar_tensor_tensor(
                        out=t2p[:], in0=sg[cc][:, 9:12, :], scalar=0.5,
                        in1=sg[cc][:, 0:3, :], op0=ALU.subtract, op1=ALU.mult,
                    )
                    t2s.append(t2p)
                c_new = []
                for cc in range(2):
                    cn = rp.tile([128, KCH, SC], F32, tag=f"c{cc}")
                    nc.vector.scalar_tensor_tensor(
                        out=cn[:], in0=t2s[cc][:], scalar=2.0, in1=t1s[cc][:],
                        op0=ALU.mult, op1=ALU.add,
                    )
                    c_new.append(cn)
                tc_t = []
                for cc in range(2):
                    tct = rp.tile([128, KCH, SC], F32, tag=f"tc{cc}")
                    nc.scalar.activation(out=tct[:], in_=c_new[cc][:], func=AF.Tanh)
                    tc_t.append(tct)
                for cc in range(2):
                    nc.vector.tensor_tensor(
                        out=hvs[cc][:, :, t, :], in0=sg[cc][:, 6:9, :], in1=tc_t[cc][:], op=ALU.mult
                    )
                rc = T - 1 - t
                for cc in range(2):
                    nc.vector.tensor_tensor(
                        out=hTr[cc][rc // QT][:, :, rc % QT, :],
                        in0=sg[cc][:, 6:9, :], in1=tc_t[cc][:], op=ALU.mult
                    )
                    c_prev[cc] = c_new[cc]
                if on_step is not None:
                    on_step(t)

    def exchange_chunk(l, j, hTr, xp_t):
        """Ship reversed-h chunk j (both chains) to the partner and gather
        the partner's chunk straight into xp_t[j] (cols = (chain, t~, b4),
        matching the own-h layout).  Gather outs must be contiguous —
        strided indirect-DMA outs write garbage (HW-verified)."""
        hq = QN // 2
        ct = ctrb[l][j].ap().rearrange("c p n -> p c n")
        nc.sync.dma_start(out=ct[:, :, 0:hq], in_=hTr[0][j][:])
        nc.sync.dma_start(out=ct[:, :, hq:QN], in_=hTr[1][j][:])
        nc.gpsimd.collective_compute(
            "AllGather",
            ALU.bypass,
            replica_groups=PAIRS,
            ins=[ctrb[l][j].ap()],
            outs=[hall[l][j].ap()],
        )
        rows = hall[l][j].ap().rearrange("r c p n -> (r c p) n")
        for cch in range(KCH):
            nc.gpsimd.indirect_dma_start(
                out=xp_t[j][:, cch, :],
                out_offset=None,
                in_=rows,
                in_offset=IndirectOffsetOnAxis(ap=gidx_sb[:, cch : cch + 1], axis=0),
            )

    def make_on_step(l, hTr, xp_t, work=None):
        work = list(work) if work else []

        def on_step(t):
            # up to one queued emission (embed ktile / G0 m-group) per step,
            # woven into the recurrence's engine slack
            if work and t % 2 == 0:
                work.pop(0)()
            # chunk j's reversed cols are complete after step T-1-QT*j
            if t >= QT - 1 and (t + 1) % QT == 0 and t != T - 1:
                j = (T - 1 - t) // QT
                exchange_chunk(l, j, hTr, xp_t)
            if t == T - 1:
                while work:
                    work.pop(0)()

        return on_step

    def exchange_last(l, hTr, xp_t):
        exchange_chunk(l, 0, hTr, xp_t)
        for cch in range(KCH):
            nc.tensor.ldweights(weights=xp_t[0][:, cch, 0:1])

    # ---- layer pipeline with scoped lifetimes (strict LIFO pools) ----
    with tc.tile_pool(name="phh", bufs=1) as phh:
        half = NT // 2
        hT0 = [phh.tile([128, KCH, half], F8E4, name=f"hT0{c}") for c in "ab"]
        hT1 = [phh.tile([128, KCH, half], F8E4, name=f"hT1{c}") for c in "ab"]
        hTr0 = [
            [phh.tile([128, KCH, QT, SCH], F8E4, name=f"hTr0{c}_{j}") for j in range(NCHUNK)]
            for c in "ab"
        ]
        hTr1 = [
            [phh.tile([128, KCH, QT, SCH], F8E4, name=f"hTr1{c}_{j}") for j in range(NCHUNK)]
            for c in "ab"
        ]
        xp_t = [phh.tile([128, KCH, QN], F8E4, name=f"xp{j}") for j in range(NCHUNK)]
        with tc.tile_pool(name="pg", bufs=1) as pgp:
            G_t = [pgp.tile([128, MCH, 512], BF16, name=f"G{nb}") for nb in range(NBG)]
            with tc.tile_pool(name="pr0", bufs=1) as pr0:
                whh0 = load_whh(0, pr0)
                with tc.tile_pool(name="pw0", bufs=1) as pw0:
                    wih0 = load_wih(0, pw0)
                    with tc.tile_pool(name="px", bufs=1) as px, tc.tile_pool(
                        name="s1", bufs=2
                    ) as s1, tc.tile_pool(
                        name="s1ps", bufs=1, space="PSUM"
                    ) as s1ps, tc.tile_pool(
                        name="g0ps", bufs=1, space="PSUM"
                    ) as g0ps:
                        xT_t = [
                            px.tile([128, DCH, 512], F8E4, name=f"xT{nb}")
                            for nb in range(NBG)
                        ]
                        embed_ktile = make_embed_ktile(s1, s1ps, xT_t)
                        # block 0 up front (the recurrence needs it at step 0);
                        # the rest weaves into the recurrence's engine slack
                        for k in range(4):
                            embed_ktile(k)
                        for m in range(MCH):
                            g0_mgroup(G_t, xT_t, wih0, g0ps, 0, m)
                        work = []
                        for nb in range(1, NBG):
                            for k in range(4 * nb, 4 * nb + 4):
                                work.append(lambda k=k: embed_ktile(k))
                            for m in range(0, MCH, 3):
                                work.append(
                                    lambda nb=nb, m=m: [
                                        g0_mgroup(G_t, xT_t, wih0, g0ps, nb, mm)
                                        for mm in range(m, m + 3)
                                    ]
                                )
                        recurrence(
                            0, G_t, hT0, hTr0, whh0,
                            on_step=make_on_step(0, hTr0, xp_t, work=work),
                        )
            exchange_last(0, hTr0, xp_t)
            with tc.tile_pool(name="pw1", bufs=1) as pw1, tc.tile_pool(
                name="g1ps", bufs=2, space="PSUM"
            ) as g1ps:
                wih1 = load_wih(1, pw1)
                nc.tensor.ldweights(weights=wih1[:, 0, 0:1])
                for nb in (1, 2, 3, 0):
                    for m in range(MCH):
                        for cc in range(2):
                            g1_mgroup(G_t, hT0, xp_t, wih1, g1ps, nb, m, cc)
            # label-only score terms (one-hot, transition pairs, start/end)
            # depend on nothing but labels: weave them into L1's engine slack
            labf_sb = phh.tile([1, NT], F32, name="labf_sb")
            ohT = phh.tile([C, NT], F32, name="ohT")
            m1T = phh.tile([C, NT], F32, name="m1T")
            s8T = phh.tile([C, 2, GB], F32, name="s8T")
            scR = phh.tile([C, 3], F32, name="scR")  # pd, st, en sums

            def sc_bcast(nb, scps):
                blk = slice(512 * nb, 512 * (nb + 1))
                bps = scps.tile([C, 512], F32, tag="scps")
                nc.tensor.matmul(
                    out=bps[:], lhsT=ones1C[:], rhs=labf_sb[:, blk],
                    start=True, stop=True,
                )
                nc.vector.tensor_copy(out=m1T[:, blk], in_=bps[:])

            def sc_oh():
                nc.vector.tensor_scalar(
                    out=ohT[:], in0=m1T[:], scalar1=iota_sb[:], scalar2=None,
                    op0=ALU.is_equal,
                )

            def sc_m1(nb, scps):
                lo = 512 * nb
                hi = min(512 * (nb + 1), NT - 8)
                mps = scps.tile([C, 512], F32, tag="scps")
                nc.tensor.matmul(
                    out=mps[:, : hi - lo], lhsT=transT_sb[:],
                    rhs=ohT[:, lo + 8 : hi + 8], start=True, stop=True,
                )
                nc.vector.tensor_copy(out=m1T[:, lo:hi], in_=mps[:, : hi - lo])

            def sc_fin():
                nc.vector.tensor_tensor(
                    out=m1T[:, : NT - 8], in0=ohT[:, : NT - 8],
                    in1=m1T[:, : NT - 8], op=ALU.mult,
                )
                nc.vector.reduce_sum(
                    out=scR[:, 0:1], in_=m1T[:, : NT - 8], axis=mybir.AxisListType.X
                )
                nc.vector.tensor_scalar_mul(
                    out=s8T[:, 0, :], in0=ohT[:, 0:GB], scalar1=stv_sb[:]
                )
                nc.vector.reduce_sum(
                    out=scR[:, 1:2], in_=s8T[:, 0, :], axis=mybir.AxisListType.X
                )
                nc.vector.tensor_scalar_mul(
                    out=s8T[:, 1, :], in0=ohT[:, NT - GB : NT], scalar1=env_sb[:]
                )
                nc.vector.reduce_sum(
                    out=scR[:, 2:3], in_=s8T[:, 1, :], axis=mybir.AxisListType.X
                )

            with tc.tile_pool(name="scps", bufs=1, space="PSUM") as scps:
                nc.sync.dma_start(out=labf_sb[:], in_=ins["labf"])
                swork = (
                    [lambda nb=nb: sc_bcast(nb, scps) for nb in range(4)]
                    + [sc_oh]
                    + [lambda nb=nb: sc_m1(nb, scps) for nb in range(4)]
                    + [sc_fin]
                )
                with tc.tile_pool(name="pr1", bufs=1) as pr1:
                    whh1 = load_whh(1, pr1)
                    recurrence(
                        1, G_t, hT1, hTr1, whh1,
                        on_step=make_on_step(1, hTr1, xp_t, work=swork),
                    )
        exchange_last(1, hTr1, xp_t)

        # ---- emissions: em^T [C, NT] = fc @ concat(h2_own, h2_partner) ----
        crf_cm = tc.tile_pool(name="crf", bufs=1)
        crf = crf_cm.__enter__()
        pe_touch_f32(cpack_sb[:, 0:1])
        tileA = crf.tile([C, NT], F32, name="tileA")  # emT
        tileB = crf.tile([C, NT], F32, name="tileB")  # Q
        tileC = crf.tile([C, NT], F32, name="tileC")  # gem
        emT = tileA
        with tc.tile_pool(name="emps", bufs=2, space="PSUM") as emps:
            nc.tensor.ldweights(weights=fcT_sb[:, 0, 0:1])
            emv8 = emT[:].rearrange("c (t b) -> c t b", b=GB)
            for nb in (1, 2, 3, 0):
                for cc in range(2):
                    ps = emps.tile([C, 256], F32, tag=f"emps{cc}")
                    # no DoubleRow: fcT's k-pair stride (C=14B) violates the
                    # dual-fp8 LDWEIGHTS stride%16 ISA restriction
                    ob = slice(256 * nb, 256 * (nb + 1))
                    pb = slice(256 * cc, 256 * (cc + 1))
                    for kk in range(DCH):
                        rhs = (
                            hT1[cc][:, kk, ob]
                            if kk < KCH
                            else xp_t[nb][:, kk - KCH, pb]
                        )
                        nc.tensor.matmul(
                            out=ps[:],
                            lhsT=fcT_sb[:, kk, :],
                            rhs=rhs,
                            start=(kk == 0),
                            stop=(kk == DCH - 1),
                        )
                    nc.vector.tensor_scalar_add(
                        out=emv8[:, 64 * nb : 64 * (nb + 1), 4 * cc : 4 * cc + 4],
                        in0=ps[:].rearrange("c (t b) -> c t b", b=SCH),
                        scalar1=fcb_sb[:],
                    )
        if DEBUG_OUTS:
            nc.sync.dma_start(out=dbg["dbg_em"], in_=emT[:])

        # ---- CRF ----
        with tc.tile_pool(name="crfw", bufs=4) as cw, tc.tile_pool(
            name="crfps", bufs=1, space="PSUM"
        ) as cps:
            Q = tileB
            nc.scalar.activation(out=Q[:], in_=emT[:], func=AF.Exp)
            dve_touch(Q[0:1, 0:1])
            Qv = Q[:].rearrange("c (t b) -> c t b", b=GB)

            # gold emissions total (pd/st/en were precomputed during L1)
            gem = tileC
            nc.vector.tensor_tensor(out=gem[:], in0=emT[:], in1=ohT[:], op=ALU.mult)
            gem_r = cw.tile([C, 1], F32, tag="gred")
            nc.vector.reduce_sum(out=gem_r[:], in_=gem[:], axis=mybir.AxisListType.X)

            score_ps = cps.tile([1, 8], F32, tag="scoreps")
            for i, r in enumerate((gem_r[:], scR[:, 0:1], scR[:, 1:2], scR[:, 2:3])):
                nc.tensor.matmul(
                    out=score_ps[:1, :1],
                    lhsT=onesC1[:],
                    rhs=r,
                    start=(i == 0),
                    stop=(i == 3),
                    skip_group_check=True,
                )
            score_sb = cw.tile([1, 1], F32, tag="scoresb")
            nc.vector.tensor_copy(out=score_sb[:], in_=score_ps[:1, :1])

            # ---- blocked forward chain in exp domain, E pre-scaled by
            # 1/ESCALE so no renorms are needed (f32 range absorbs the
            # drift; the log-correction is added at the end).
            # Block 0 runs the alpha chain over t in [0, BT); blocks k=1..3
            # run transfer-matrix chains Yk = M_k^T = E'D_lo ... E'D_hi
            # (built high-t to low-t), all NBLK chains concurrently.
            expTTp_bf = cw.tile([C, C], BF16, tag="ettbf")
            nc.vector.tensor_copy(out=expTTp_bf[:], in_=expTTp_f32)
            v_prev = cw.tile([C, GB], F32, tag="v")
            nc.vector.tensor_scalar_mul(out=v_prev[:], in0=Qv[:, 0, :], scalar1=expst_sb[:])
            eye_bc = eye_sb.unsqueeze(1).broadcast_to([C, GB, C])
            yps_prev = [None] * NBLK
            for s in range(BT):
                if s > 0:
                    vps = cps.tile([C, GB], F32, tag="vps")
                    nc.tensor.matmul(out=vps[:], lhsT=E_sb[:], rhs=v_prev[:], start=True, stop=True)
                    v_new = cw.tile([C, GB], F32, tag="v")
                    nc.vector.tensor_tensor(out=v_new[:], in0=vps[:], in1=Qv[:, s, :], op=ALU.mult)
                    v_prev = v_new
                for k in range(1, NBLK):
                    t = BT * (k + 1) - 1 - s
                    qb = Qv[:, t, :].unsqueeze(2).broadcast_to([C, GB, C])
                    w = cw.tile([C, GB, C], BF16, tag=f"w{k}")
                    if s == 0:
                        nc.vector.tensor_tensor(out=w[:], in0=eye_bc, in1=qb, op=ALU.mult)
                    else:
                        nc.vector.tensor_tensor(
                            out=w[:],
                            in0=yps_prev[k][:].rearrange("c (b j) -> c b j", b=GB),
                            in1=qb,
                            op=ALU.mult,
                        )
                    yp = cps.tile([C, GB * C], F32, tag=f"yps{k}")
                    nc.tensor.matmul(out=yp[:], lhsT=expTTp_bf[:], rhs=w[:], start=True, stop=True)
                    yps_prev[k] = yp
            yfin = []
            for k in range(1, NBLK):
                yf = cw.tile([C, GB, C], BF16, tag=f"yf{k}")
                nc.vector.tensor_copy(
                    out=yf[:], in_=yps_prev[k][:].rearrange("c (b j) -> c b j", b=GB)
                )
                yfin.append(yf)
            # fold block results into the alpha vector
            for k in range(1, NBLK):
                vb = cw.tile([C, GB], BF16, tag="vb")
                nc.vector.tensor_copy(out=vb[:], in_=v_prev[:])
                aps = cps.tile([C, GB], F32, tag="vps")
                for b in range(GB):
                    nc.tensor.matmul(
                        out=aps[:, b : b + 1],
                        lhsT=yfin[k - 1][:, b, :],
                        rhs=vb[:, b : b + 1],
                        start=(b == 0),
                        stop=(b == GB - 1),
                        skip_group_check=True,
                    )
                v_new = cw.tile([C, GB], F32, tag="v")
                nc.vector.tensor_copy(out=v_new[:], in_=aps[:])
                v_prev = v_new
            vend = cw.tile([C, GB], F32, tag="vend")
            nc.vector.tensor_scalar_mul(out=vend[:], in0=v_prev[:], scalar1=expen_sb[:])
            zps = cps.tile([1, GB], F32, tag="cps1")
            nc.tensor.matmul(out=zps[:], lhsT=onesC1[:], rhs=vend[:], start=True, stop=True)
            lnz = cw.tile([1, GB], F32, tag="lnz")
            nc.scalar.activation(out=lnz[:], in_=zps[:], func=AF.Ln)
            logz = cw.tile([1, GB], F32, tag="logz")
            nc.vector.tensor_scalar(
                out=logz[:], in0=lnz[:], scalar1=float((T - 1) * np.log(ESCALE)),
                scalar2=None, op0=ALU.add,
            )
            lz_tot = cw.tile([1, 1], F32, tag="lztot")
            nc.vector.reduce_sum(out=lz_tot[:], in_=logz[:], axis=mybir.AxisListType.X)
            loss_sb = cw.tile([1, 1], F32, tag="loss_sb")
            nc.vector.tensor_tensor(out=loss_sb[:], in0=lz_tot[:], in1=score_sb[:], op=ALU.subtract)
            nc.sync.dma_start(out=loss_out, in_=loss_sb[:])
            if DEBUG_OUTS:
                dsc = cw.tile([1, 2], F32, tag="dsc")
                nc.vector.tensor_copy(out=dsc[:, 0:1], in_=lz_tot[:])
                nc.vector.tensor_copy(out=dsc[:, 1:2], in_=score_sb[:])
                nc.sync.dma_start(out=dbg["dbg_sc"], in_=dsc[:])
        crf_cm.__exit__(None, None, None)

    est.close()


# ---------------------------------------------------------------------------
# host side
# ---------------------------------------------------------------------------

def make_in_maps(inputs):
    ids = np.asarray(inputs["input_ids"]).astype(np.int64)
    labels = np.asarray(inputs["labels"]).astype(np.int64)
    word_emb = _f32(inputs["word_emb"])
    pos_emb = _f32(inputs["pos_emb"])
    type_emb = _f32(inputs["type_emb"])
    ln_g = _f32(inputs["ln_g"])
    ln_b = _f32(inputs["ln_b"])
    w_ih = _f32(inputs["w_ih"])
    w_hh = _f32(inputs["w_hh"])
    b_ih = _f32(inputs["b_ih"])
    b_hh = _f32(inputs["b_hh"])
    fc_w = _f32(inputs["fc_w"])
    fc_b = _f32(inputs["fc_b"])
    crf_start = _f32(inputs["crf_start"])
    crf_end = _f32(inputs["crf_end"])
    crf_trans = _f32(inputs["crf_trans"])

    posty0 = pos_emb[:T] + type_emb[0][None, :]
    word_emb_bf = _bf(word_emb)

    in_maps = []
    for core in range(NCORES):
        g, d = core // 2, core % 2
        sl = slice(GB * g, GB * (g + 1))
        ids_loc = ids[sl]
        lab_loc = labels[sl]
        posty = posty0
        if d == 1:
            ids_loc = ids_loc[:, ::-1]
            lab_loc = lab_loc[:, ::-1]
            posty = posty0[::-1]

        # layer-0 weights with LN affine folded in (gate-permuted, g x2)
        w0 = _perm_gates(w_ih[0, d] * ln_g[None, :])
        bias0 = _perm_gates((b_ih[0, d] + b_hh[0, d] + w_ih[0, d] @ ln_b)[:, None])[:, 0]
        # layer-1 weights, columns permuted to local [own, partner] order
        w1 = w_ih[1, d]
        if d == 1:
            w1 = np.concatenate([w1[:, HD:], w1[:, :HD]], axis=1)
        w1 = _perm_gates(w1)
        bias1 = _perm_gates((b_ih[1, d] + b_hh[1, d])[:, None])[:, 0]
        fcp = fc_w if d == 0 else np.concatenate([fc_w[:, HD:], fc_w[:, :HD]], axis=1)

        trans_eff = crf_trans if d == 0 else crf_trans.T
        start_eff = crf_start if d == 0 else crf_end
        end_eff = crf_end if d == 0 else crf_start

        pr = 1 - d
        gidx = np.empty((128, KCH), np.int32)
        for cch in range(KCH):
            gidx[:, cch] = pr * (KCH * 128) + cch * 128 + np.arange(128)

        cpack = np.zeros((C, CPW), np.float32)
        cpack[:, 0:C] = np.exp(trans_eff) / ESCALE
        cpack[:, C : 2 * C] = trans_eff.T
        cpack[:, 28] = np.exp(start_eff)
        cpack[:, 29] = np.exp(end_eff)
        cpack[:, 30] = start_eff
        cpack[:, 31] = end_eff
        cpack[:, 32] = np.arange(C, dtype=np.float32)
        cpack[:, 33] = fc_b
        cpack[:, 34 : 34 + C] = (np.exp(trans_eff) / ESCALE).T
        cpack[:, 34 + C : 34 + 2 * C] = np.eye(C, dtype=np.float32)

        b01 = np.concatenate(
            [bias0.reshape(MCH, 128).T, bias1.reshape(MCH, 128).T], axis=1
        )

        # t-major token order: token n = (t, b)
        ids_tm = np.ascontiguousarray(ids_loc.T).reshape(NT, 1)
        lab_tm = np.ascontiguousarray(lab_loc.T).reshape(1, NT)
        posty_rep = np.repeat(np.asarray(posty, np.float32), GB, axis=0)
        in_maps.append(
            dict(
                ids32=np.ascontiguousarray(ids_tm.astype(np.int32)),
                labf=np.ascontiguousarray(lab_tm.astype(np.float32)),
                word_emb=word_emb_bf,
                posty=_bf(posty_rep),
                wih0T=_f8(w0.T),
                wih1T=_f8(w1.T),
                whh0T=_f8(_perm_gates(w_hh[0, d]).T),
                whh1T=_f8(_perm_gates(w_hh[1, d]).T),
                b01=np.ascontiguousarray(b01),
                fcT=_f8(fcp.T),
                cpack=cpack,
                gidx=gidx,
            )
        )
    return in_maps


_PROGRAM = None
_COST_MODEL_NS = None


def _get_program():
    global _PROGRAM, _COST_MODEL_NS
    if _PROGRAM is None:
        _PROGRAM = build_program()
        try:
            from concourse.timeline_sim import TimelineSim

            _COST_MODEL_NS = int(TimelineSim(_PROGRAM, trace=False, no_exec=True).simulate())
        except Exception:
            _COST_MODEL_NS = None
    return _PROGRAM


def run(inputs, trace=False):
    nc = _get_program()
    in_maps = make_in_maps(inputs)
    res = run_bass_kernel_spmd(nc, in_maps, core_ids=list(range(NCORES)), trace=trace)
    total = np.float64(0.0)
    for g in range(4):
        total += np.float64(res.results[2 * g]["loss"][0, 0])
    return np.asarray(total, dtype=np.float32), res


def kernel(**inputs):
    out, _ = run(inputs, trace=False)
    return out



# revision 69
# speedup vs baseline: 1.0468x; 1.0185x over previous
"""BiLSTM-CRF sequence-tagging loss on 8 Trainium2 NeuronCores.

Sharding: 8 cores = 4 batch-groups x 2 LSTM directions.
  core 2g+d handles sequences [8g, 8g+8) ; d=0 forward, d=1 backward.
Backward cores receive time-reversed inputs (ids/pos/labels), so one SPMD
program runs on all cores; their CRF uses transposed transitions with
start/end swapped (same loss by path reversal), and their layer-2/emission
weights are column-permuted so the local [own_h, partner_h] concat order is
uniform.  The h-streams are exchanged pairwise via AllGather through DRAM;
the partner slot is fetched with an indirect-DMA row gather whose indices
are per-core input data (keeps the program core-uniform).
"""

import os
import sys

import numpy as np

for _p in ("/opt/trn_rl_repo", "/root/.axon_site/_ro/trn_rl_repo"):
    if os.path.isdir(_p) and _p not in sys.path:
        sys.path.insert(0, _p)

import ml_dtypes  # noqa: E402

import concourse.bass as bass  # noqa: E402
import concourse.bacc as bacc  # noqa: E402
import concourse.tile as tile  # noqa: E402
from concourse import mybir  # noqa: E402
from concourse.bass import IndirectOffsetOnAxis  # noqa: E402
from concourse.bass_utils import run_bass_kernel_spmd  # noqa: E402
from concourse.masks import make_identity  # noqa: E402

F32 = mybir.dt.float32
BF16 = mybir.dt.bfloat16
F8E4 = mybir.dt.float8e4
I32 = mybir.dt.int32
AF = mybir.ActivationFunctionType
ALU = mybir.AluOpType

# problem shapes (hardcoded per contract)
B, T, V, D, C, HD = 32, 256, 30522, 768, 14, 384
L = 2
NCORES = 8
GB = 8            # sequences per core group
NT = GB * T       # tokens per core = 2048
NTILE = NT // 128  # 16
MCH = 12          # gate chunks of 128 (4*HD/128)
KCH = 3           # hidden chunks (HD/128)
DCH = 6           # input-dim chunks (D/128)
LN_EPS = 1e-12
PAIRS = [[0, 1], [2, 3], [4, 5], [6, 7]]
ESCALE = 16.0     # folded per-step scaling of exp(trans); no renorms needed
NBLK = 4          # CRF scan blocks (1 alpha chain + NBLK-1 matrix chains)
BT = T // NBLK    # 64 steps per block
CPW = 34 + 2 * 14  # cpack width
NCHUNK = 4        # exchange chunks per layer
QT = T // NCHUNK  # 64 timesteps per chunk
QN = GB * QT      # 512 columns per chunk
SCH = GB // 2     # sequences per recurrence chain
NBG = NT // 512   # 512-token (64-step) blocks

DEBUG_OUTS = False


def _bf(x):
    return np.ascontiguousarray(np.asarray(x, dtype=np.float32)).astype(ml_dtypes.bfloat16)


def _f32(x):
    return np.ascontiguousarray(np.asarray(x, dtype=np.float32))


def _f8(x):
    return np.ascontiguousarray(np.asarray(x, dtype=np.float32)).astype(
        ml_dtypes.float8_e4m3
    )


def _perm_gates(w):
    """torch gate order i,f,g,o -> device order i,f,o,g with the g block
    scaled by 2 (tanh(x) = 2*sigmoid(2x)-1)."""
    H = HD
    return np.concatenate([w[0:H], w[H:2 * H], w[3 * H:4 * H], 2.0 * w[2 * H:3 * H]], axis=0)


# ---------------------------------------------------------------------------
# device program
# ---------------------------------------------------------------------------

def build_program():
    nc = bacc.Bacc("TRN2", target_bir_lowering=False, debug=False, num_devices=NCORES)

    def din(name, shape, dt):
        return nc.dram_tensor(name, shape, dt, kind="ExternalInput").ap()

    ins = dict(
        ids32=din("ids32", [NT, 1], I32),
        labf=din("labf", [1, NT], F32),
        word_emb=din("word_emb", [V, D], BF16),
        posty=din("posty", [NT, D], BF16),
        wih0T=din("wih0T", [D, 4 * HD], F8E4),
        wih1T=din("wih1T", [D, 4 * HD], F8E4),
        whh0T=din("whh0T", [HD, 4 * HD], F8E4),
        whh1T=din("whh1T", [HD, 4 * HD], F8E4),
        b01=din("b01", [128, 2 * MCH], F32),
        fcT=din("fcT", [D, C], F8E4),
        cpack=din("cpack", [C, CPW], F32),
        gidx=din("gidx", [128, KCH], I32),
    )

    loss_out = nc.dram_tensor("loss", [1, 1], F32, kind="ExternalOutput").ap()
    dbg = {}
    if DEBUG_OUTS:
        dbg["dbg_xt"] = nc.dram_tensor("dbg_xt", [128, DCH, NT], BF16, kind="ExternalOutput").ap()
        dbg["dbg_g"] = nc.dram_tensor("dbg_g", [128, MCH, NT], BF16, kind="ExternalOutput").ap()
        dbg["dbg_h1"] = nc.dram_tensor("dbg_h1", [128, KCH, NT], BF16, kind="ExternalOutput").ap()
        dbg["dbg_h2"] = nc.dram_tensor("dbg_h2", [128, KCH, NT], BF16, kind="ExternalOutput").ap()
        dbg["dbg_em"] = nc.dram_tensor("dbg_em", [C, NT], F32, kind="ExternalOutput").ap()
        dbg["dbg_sc"] = nc.dram_tensor("dbg_sc", [1, 2], F32, kind="ExternalOutput").ap()

    # internal DRAM for pairwise exchange (fp8, 4 time-chunks per layer so
    # collectives overlap the recurrence)
    ctrb = [
        [nc.dram_tensor(f"ctrb{l}_{j}", [KCH, 128, QN], F8E4) for j in range(NCHUNK)]
        for l in range(L)
    ]
    hall = [
        [nc.dram_tensor(f"hall{l}_{j}", [2, KCH, 128, QN], F8E4) for j in range(NCHUNK)]
        for l in range(L)
    ]

    with tile.TileContext(nc) as tc:
        _build_body(tc, ins, loss_out, dbg, ctrb, hall)

    nc.compile()
    return nc


def _build_body(tc, ins, loss_out, dbg, ctrb, hall):
    nc = tc.nc
    from contextlib import ExitStack

    est = ExitStack()
    pers = est.enter_context(tc.tile_pool(name="pers", bufs=1))

    # ---- persistent SBUF state (small constants only) ----
    def load_wih(l, pool):
        wt = pool.tile([128, DCH, 4 * HD], F8E4, name=f"wih{l}")
        src = ins["wih0T"] if l == 0 else ins["wih1T"]
        nc.sync.dma_start(out=wt[:], in_=src.rearrange("(k p) m -> p k m", p=128))
        return wt

    def load_whh(l, pool):
        ht = pool.tile([128, KCH, 4 * HD], F8E4, name=f"whh{l}")
        src = ins["whh0T"] if l == 0 else ins["whh1T"]
        nc.sync.dma_start(out=ht[:], in_=src.rearrange("(k p) m -> p k m", p=128))
        return ht

    # scratch + absorbers: this toolchain allows only ONE sem wait per
    # instruction, so every junction of two producers gets a tiny absorber op
    # that folds one producer into the consuming engine's clock first.
    scr_dve = pers.tile([1, 4], F32, name="scr_dve")
    scr_gp = pers.tile([1, 4], I32, name="scr_gp")
    pabs = est.enter_context(tc.tile_pool(name="pabs", bufs=1, space="PSUM"))
    pscr = pabs.tile([1, 8], F32, name="pscr")

    def dve_touch(ap):
        nc.vector.tensor_copy(out=scr_dve[:, 0:1], in_=ap)

    def pe_touch_f32(ap_col):
        nc.tensor.matmul(out=pscr[:1, :1], lhsT=ap_col, rhs=ap_col, start=True, stop=True)

    b_sb = pers.tile([128, 2 * MCH], F32, name="b_sb")
    nc.sync.dma_start(out=b_sb[:], in_=ins["b01"])
    dve_touch(b_sb[0:1, 0:1])

    fcT_sb = pers.tile([128, DCH, C], F8E4, name="fcT")
    nc.sync.dma_start(out=fcT_sb[:], in_=ins["fcT"].rearrange("(k p) m -> p k m", p=128))

    cpack_sb = pers.tile([C, CPW], F32, name="cpack_sb")
    nc.sync.dma_start(out=cpack_sb[:], in_=ins["cpack"])
    dve_touch(cpack_sb[0:1, 0:1])
    E_sb = cpack_sb[:, 0:C]  # exp(trans)/ESCALE
    transT_sb = cpack_sb[:, C : 2 * C]
    expst_sb = cpack_sb[:, 28:29]
    expen_sb = cpack_sb[:, 29:30]
    stv_sb = cpack_sb[:, 30:31]
    env_sb = cpack_sb[:, 31:32]
    iota_sb = cpack_sb[:, 32:33]
    fcb_sb = cpack_sb[:, 33:34]
    expTTp_f32 = cpack_sb[:, 34 : 34 + C]  # exp(trans.T)/ESCALE
    eye_sb = cpack_sb[:, 34 + C : 34 + 2 * C]  # identity

    gidx_sb = pers.tile([128, KCH], I32, name="gidx_sb")
    nc.sync.dma_start(out=gidx_sb[:], in_=ins["gidx"])
    nc.gpsimd.tensor_copy(out=scr_gp[:, 0:1], in_=gidx_sb[0:1, 0:1])

    ids_sb = pers.tile([128, NTILE], I32, name="ids_sb")
    nc.sync.dma_start(out=ids_sb[:], in_=ins["ids32"].rearrange("(k p) o -> p (k o)", p=128))

    ident = pers.tile([128, 128], F32, name="ident")
    make_identity(nc, ident[:])
    ident_bf = pers.tile([128, 128], BF16, name="ident_bf")
    nc.vector.tensor_copy(out=ident_bf[:], in_=ident[:])
    pe_touch_f32(ident[:, 0:1])
    eps_sb = pers.tile([128, 1], F32, name="eps_sb")
    nc.vector.memset(eps_sb[:], LN_EPS)
    ones1C = pers.tile([1, C], F32, name="ones1C")
    nc.vector.memset(ones1C[:], 1.0)
    onesC1 = pers.tile([C, 1], F32, name="onesC1")
    nc.vector.memset(onesC1[:], 1.0)

    # ---- helpers ----
    def make_embed_ktile(s1, s1ps, xT_t):
        """Returns emit(k): embeds token k-tile k (128 t-major tokens) into
        xT_t[k//4]. Called lazily so ktiles can be woven into the layer-0
        recurrence's engine slack."""

        def emit(k):
            posty_sb = s1.tile([128, D], BF16, tag="posty")
            nc.sync.dma_start(
                out=posty_sb[:], in_=ins["posty"][128 * k : 128 * (k + 1), :]
            )
            emb = s1.tile([128, D], BF16, tag="emb")
            nc.gpsimd.indirect_dma_start(
                out=emb[:],
                out_offset=None,
                in_=ins["word_emb"],
                in_offset=IndirectOffsetOnAxis(ap=ids_sb[:, k : k + 1], axis=0),
            )
            emb2 = s1.tile([128, D], BF16, tag="emb2")
            nc.vector.tensor_add(out=emb2[:], in0=emb[:], in1=posty_sb[:])
            stats = s1.tile([128, 3, 6], F32, tag="stats")
            embv = emb2[:].rearrange("p (s q) -> p s q", s=3)
            for sg in range(3):
                nc.vector.bn_stats(out=stats[:, sg, :], in_=embv[:, sg, :])
            mv = s1.tile([128, 2], F32, tag="mv")
            nc.vector.bn_aggr(out=mv[:], in_=stats[:])
            std = s1.tile([128, 1], F32, tag="std")
            nc.scalar.activation(out=std[:], in_=mv[:, 1:2], func=AF.Sqrt, bias=eps_sb[:])
            rstd = s1.tile([128, 1], F32, tag="rstd")
            nc.vector.reciprocal(out=rstd[:], in_=std[:])
            xln = s1.tile([128, D], BF16, tag="xln")
            nc.vector.tensor_scalar(
                out=xln[:],
                in0=emb2[:],
                scalar1=mv[:, 0:1],
                scalar2=rstd[:],
                op0=ALU.subtract,
                op1=ALU.mult,
            )
            for j in range(DCH):
                tp = s1ps.tile([128, 128], BF16, tag="tp")
                nc.tensor.transpose(
                    out=tp[:], in_=xln[:, 128 * j : 128 * (j + 1)], identity=ident_bf[:]
                )
                # spread PSUM->SBUF copies across engines; DVE is the
                # embed bottleneck
                dst = xT_t[k // 4][:, j, 128 * (k % 4) : 128 * (k % 4 + 1)]
                if j % 2 == 0:
                    nc.vector.tensor_copy(out=dst, in_=tp[:])
                else:
                    nc.scalar.copy(out=dst, in_=tp[:])

        return emit

    DR = mybir.MatmulPerfMode.DoubleRow

    def g0_mgroup(G_t, xT_t, wih, gps, nb, m):
        """One gate-chunk m of G0 block nb: 3 fp8 DoubleRow matmuls + bias."""
        ms = slice(128 * m, 128 * (m + 1))
        ps = gps.tile([128, 512], F32, tag="gps")
        for kp in range(DCH // 2):
            nc.tensor.matmul(
                out=ps[:],
                lhsT=wih[:, 2 * kp : 2 * kp + 2, ms],
                rhs=xT_t[nb][:, 2 * kp : 2 * kp + 2, :],
                start=(kp == 0),
                stop=(kp == DCH // 2 - 1),
                perf_mode=DR,
            )
        nc.vector.tensor_scalar_add(
            out=G_t[nb][:, m, :], in0=ps[:], scalar1=b_sb[:, m : m + 1]
        )

    def cat_segs(wT, hT, xp_t, nb, cc, ms):
        """Segments for concat(h_own[3 chunks], xp[3 chunks]) for chain cc of
        block nb: pairs (0,1)/(4,5) ride DoubleRow, 2/3 go single (they
        straddle the own/partner tile boundary)."""
        ob = slice(256 * nb, 256 * (nb + 1))
        pb = slice(256 * cc, 256 * (cc + 1))
        return [
            (wT[:, 0:2, ms], hT[cc][:, 0:2, ob], DR),
            (wT[:, 2, ms], hT[cc][:, 2, ob], None),
            (wT[:, 3, ms], xp_t[nb][:, 0, pb], None),
            (wT[:, 4:6, ms], xp_t[nb][:, 1:3, pb], DR),
        ]

    def g1_mgroup(G_t, hT, xp_t, wih, gps, nb, m, cc):
        """One (gate-chunk m, chain cc) piece of G1 block nb."""
        ms = slice(128 * m, 128 * (m + 1))
        ps = gps.tile([128, 256], F32, tag=f"gps{cc}")
        segs = cat_segs(wih, hT, xp_t, nb, cc, ms)
        for si, (lh, rhs, perf) in enumerate(segs):
            nc.tensor.matmul(
                out=ps[:], lhsT=lh, rhs=rhs,
                start=(si == 0), stop=(si == len(segs) - 1), perf_mode=perf,
            )
        gv8 = G_t[nb][:, m, :].rearrange("p (t b) -> p t b", b=GB)
        nc.vector.tensor_scalar_add(
            out=gv8[:, :, 4 * cc : 4 * cc + 4],
            in0=ps[:].rearrange("p (t b) -> p t b", b=SCH),
            scalar1=b_sb[:, MCH + m : MCH + m + 1],
        )

    def recurrence(l, G_t, hT, hTr, whh, on_step=None, psbufs=2):
        """LSTM scan over time (t-major token layout); writes hT (pair of
        per-chain tiles, cols (t, b4)) and hTr (pair of per-chain lists of
        per-chunk fp8 tiles, time-reversed).

        Gates in device order (i,f,o,g) with g pre-scaled x2: one Sigmoid
        covers every gate (tanh(x) = 2*sigmoid(2x)-1).  G_t is injected into
        the gate PSUM by an identity matmul; one accumulation group per tile
        (start=True zeroes the whole 2KB bank region).

        The two chains use SEPARATE h tiles: the tile framework tracks deps
        at tile granularity, so a shared tile would make chain 0's matmul
        reads wait on chain 1's h writes and serialize the stagger."""
        Gv8 = [g[:].rearrange("p m (t b) -> p m t b", b=GB) for g in G_t]
        hvs = [h[:].rearrange("p c (t b) -> p c t b", b=SCH) for h in hT]
        with tc.tile_pool(name=f"r{l}", bufs=8) as rp, tc.tile_pool(
            name=f"r{l}c", bufs=1) as rcp, tc.tile_pool(
            name=f"r{l}ps", bufs=psbufs, space="PSUM"
        ) as rps:
            nc.tensor.ldweights(weights=whh[:, 0, 0:1])
            SC = SCH  # two sliding chains of 4 sequences
            c_prev = []
            for cc in range(2):
                cz = rcp.tile([128, KCH, SC], F32, name=f"c0_{l}_{cc}")
                nc.vector.memset(cz[:], 0.0)
                c_prev.append(cz)
            # per-step emission is grouped by engine stage (both chains
            # adjacent) so each engine's in-order queue never has an
            # instruction whose wait blocks the other chain's work.
            for t in range(T):
                gv = Gv8[t // 64][:, :, t % 64, :]  # [128, MCH, 8]
                ps = []
                for cc in range(2):
                    bs = slice(SC * cc, SC * (cc + 1))
                    p = rps.tile([128, MCH, SC], F32, tag=f"ps{cc}")
                    ps.append(p)
                    if t == 0:
                        nc.tensor.matmul(
                            out=p[:], lhsT=ident_bf[:], rhs=gv[:, :, bs],
                            start=True, stop=True, skip_group_check=True,
                        )
                    else:
                        # G-inject first: it has no h dependency, so it soaks
                        # up the stale PSUM-reuse wait and the first whh
                        # matmul carries only the live h wait.
                        nc.tensor.matmul(
                            out=p[:], lhsT=ident_bf[:], rhs=gv[:, :, bs],
                            start=True, stop=False, skip_group_check=True,
                        )
                        for m in range(MCH):
                            ms = slice(128 * m, 128 * (m + 1))
                            nc.tensor.matmul(
                                out=p[:, m, :],
                                lhsT=whh[:, 0:2, ms],
                                rhs=hvs[cc][:, 0:2, t - 1, :],
                                start=False,
                                stop=False,
                                perf_mode=DR,
                                skip_group_check=True,
                            )
                            nc.tensor.matmul(
                                out=p[:, m, :],
                                lhsT=whh[:, 2, ms],
                                rhs=hvs[cc][:, 2, t - 1, :],
                                start=False,
                                stop=(m == MCH - 1),
                                skip_group_check=True,
                            )
                sg = []
                for cc in range(2):
                    s = rp.tile([128, MCH, SC], F32, tag=f"sg{cc}")
                    nc.scalar.activation(out=s[:], in_=ps[cc][:], func=AF.Sigmoid)
                    sg.append(s)
                t1s, t2s = [], []
                for cc in range(2):
                    # c = f*c_prev + i*(2*sg_g - 1); t2p/c fused via stt
                    t1 = rp.tile([128, KCH, SC], F32, tag=f"t1{cc}")
                    nc.vector.tensor_tensor(
                        out=t1[:], in0=sg[cc][:, 3:6, :], in1=c_prev[cc][:], op=ALU.mult
                    )
                    t1s.append(t1)
                for cc in range(2):
                    t2p = rp.tile([128, KCH, SC], F32, tag=f"t2{cc}")
                    nc.vector.scalar_tensor_tensor(
                        out=t2p[:], in0=sg[cc][:, 9:12, :], scalar=0.5,
                        in1=sg[cc][:, 0:3, :], op0=ALU.subtract, op1=ALU.mult,
                    )
                    t2s.append(t2p)
                c_new = []
                for cc in range(2):
                    cn = rp.tile([128, KCH, SC], F32, tag=f"c{cc}")
                    nc.vector.scalar_tensor_tensor(
                        out=cn[:], in0=t2s[cc][:], scalar=2.0, in1=t1s[cc][:],
                        op0=ALU.mult, op1=ALU.add,
                    )
                    c_new.append(cn)
                tc_t = []
                for cc in range(2):
                    tct = rp.tile([128, KCH, SC], F32, tag=f"tc{cc}")
                    nc.scalar.activation(out=tct[:], in_=c_new[cc][:], func=AF.Tanh)
                    tc_t.append(tct)
                for cc in range(2):
                    nc.vector.tensor_tensor(
                        out=hvs[cc][:, :, t, :], in0=sg[cc][:, 6:9, :], in1=tc_t[cc][:], op=ALU.mult
                    )
                rc = T - 1 - t
                for cc in range(2):
                    nc.vector.tensor_tensor(
                        out=hTr[cc][rc // QT][:, :, rc % QT, :],
                        in0=sg[cc][:, 6:9, :], in1=tc_t[cc][:], op=ALU.mult
                    )
                    c_prev[cc] = c_new[cc]
                if on_step is not None:
                    on_step(t)

    def exchange_chunk(l, j, hTr, xp_t):
        """Ship reversed-h chunk j (both chains) to the partner and gather
        the partner's chunk straight into xp_t[j] (cols = (chain, t~, b4),
        matching the own-h layout).  Gather outs must be contiguous —
        strided indirect-DMA outs write garbage (HW-verified)."""
        hq = QN // 2
        ct = ctrb[l][j].ap().rearrange("c p n -> p c n")
        nc.sync.dma_start(out=ct[:, :, 0:hq], in_=hTr[0][j][:])
        nc.sync.dma_start(out=ct[:, :, hq:QN], in_=hTr[1][j][:])
        nc.gpsimd.collective_compute(
            "AllGather",
            ALU.bypass,
            replica_groups=PAIRS,
            ins=[ctrb[l][j].ap()],
            outs=[hall[l][j].ap()],
        )
        rows = hall[l][j].ap().rearrange("r c p n -> (r c p) n")
        for cch in range(KCH):
            nc.gpsimd.indirect_dma_start(
                out=xp_t[j][:, cch, :],
                out_offset=None,
                in_=rows,
                in_offset=IndirectOffsetOnAxis(ap=gidx_sb[:, cch : cch + 1], axis=0),
            )

    def make_on_step(l, hTr, xp_t, work=None, work2=None):
        work = list(work) if work else []
        work2 = list(work2) if work2 else []

        def on_step(t):
            # up to one queued emission (embed ktile / G0 m-group) per step,
            # woven into the recurrence's engine slack
            if work and t % 2 == 0:
                work.pop(0)()
            # chunk j's reversed cols are complete after step T-1-QT*j
            if t >= QT - 1 and (t + 1) % QT == 0 and t != T - 1:
                j = (T - 1 - t) // QT
                exchange_chunk(l, j, hTr, xp_t)
            # late work (G1 blocks 1-2): every input emitted by step 191
            if work2 and t >= 3 * QT:
                work2.pop(0)()
            if t == T - 1:
                while work:
                    work.pop(0)()
                while work2:
                    work2.pop(0)()

        return on_step

    def exchange_last(l, hTr, xp_t):
        exchange_chunk(l, 0, hTr, xp_t)
        for cch in range(KCH):
            nc.tensor.ldweights(weights=xp_t[0][:, cch, 0:1])

    # ---- layer pipeline with scoped lifetimes (strict LIFO pools) ----
    with tc.tile_pool(name="phh", bufs=1) as phh:
        half = NT // 2
        hT0 = [phh.tile([128, KCH, half], F8E4, name=f"hT0{c}") for c in "ab"]
        hT1 = [phh.tile([128, KCH, half], F8E4, name=f"hT1{c}") for c in "ab"]
        hTr0 = [
            [phh.tile([128, KCH, QT, SCH], F8E4, name=f"hTr0{c}_{j}") for j in range(NCHUNK)]
            for c in "ab"
        ]
        hTr1 = [
            [phh.tile([128, KCH, QT, SCH], F8E4, name=f"hTr1{c}_{j}") for j in range(NCHUNK)]
            for c in "ab"
        ]
        xp_t = [phh.tile([128, KCH, QN], F8E4, name=f"xp{j}") for j in range(NCHUNK)]
        with tc.tile_pool(name="pg", bufs=1) as pgp:
            G_t = [pgp.tile([128, MCH, 512], BF16, name=f"G{nb}") for nb in range(NBG)]
            with tc.tile_pool(name="pw1", bufs=1) as pw1, tc.tile_pool(
                name="g1ps", bufs=1, space="PSUM"
            ) as g1ps:
              wih1 = load_wih(1, pw1)
              with tc.tile_pool(name="pr0", bufs=1) as pr0:
                whh0 = load_whh(0, pr0)
                with tc.tile_pool(name="pw0", bufs=1) as pw0:
                    wih0 = load_wih(0, pw0)
                    with tc.tile_pool(name="px", bufs=1) as px, tc.tile_pool(
                        name="s1", bufs=2
                    ) as s1, tc.tile_pool(
                        name="s1ps", bufs=1, space="PSUM"
                    ) as s1ps, tc.tile_pool(
                        name="g0ps", bufs=1, space="PSUM"
                    ) as g0ps:
                        xT_t = [
                            px.tile([128, DCH, 512], F8E4, name=f"xT{nb}")
                            for nb in range(NBG)
                        ]
                        embed_ktile = make_embed_ktile(s1, s1ps, xT_t)
                        # block 0 up front (the recurrence needs it at step 0);
                        # the rest weaves into the recurrence's engine slack
                        for k in range(4):
                            embed_ktile(k)
                        for m in range(MCH):
                            g0_mgroup(G_t, xT_t, wih0, g0ps, 0, m)
                        work = []
                        for nb in range(1, NBG):
                            for k in range(4 * nb, 4 * nb + 4):
                                work.append(lambda k=k: embed_ktile(k))
                            for m in range(0, MCH, 3):
                                work.append(
                                    lambda nb=nb, m=m: [
                                        g0_mgroup(G_t, xT_t, wih0, g0ps, nb, mm)
                                        for mm in range(m, m + 3)
                                    ]
                                )
                        # G1 blocks 1-2: own h emitted by step 127, xp gather
                        # emitted at step 191 -> weave from step 192 on
                        work2 = [
                            lambda nb=nb, m=m, cc=cc: g1_mgroup(
                                G_t, hT0, xp_t, wih1, g1ps, nb, m, cc
                            )
                            for nb in (1, 2)
                            for m in range(MCH)
                            for cc in range(2)
                        ]
                        recurrence(
                            0, G_t, hT0, hTr0, whh0,
                            on_step=make_on_step(0, hTr0, xp_t, work=work, work2=work2),
                            psbufs=1,
                        )
              exchange_last(0, hTr0, xp_t)
              for nb in (3, 0):
                    for m in range(MCH):
                        for cc in range(2):
                            g1_mgroup(G_t, hT0, xp_t, wih1, g1ps, nb, m, cc)
            # label-only score terms (one-hot, transition pairs, start/end)
            # depend on nothing but labels: weave them into L1's engine slack
            labf_sb = phh.tile([1, NT], F32, name="labf_sb")
            ohT = phh.tile([C, NT], F32, name="ohT")
            m1T = phh.tile([C, NT], F32, name="m1T")
            s8T = phh.tile([C, 2, GB], F32, name="s8T")
            scR = phh.tile([C, 3], F32, name="scR")  # pd, st, en sums

            def sc_bcast(nb, scps):
                blk = slice(512 * nb, 512 * (nb + 1))
                bps = scps.tile([C, 512], F32, tag="scps")
                nc.tensor.matmul(
                    out=bps[:], lhsT=ones1C[:], rhs=labf_sb[:, blk],
                    start=True, stop=True,
                )
                nc.vector.tensor_copy(out=m1T[:, blk], in_=bps[:])

            def sc_oh():
                nc.vector.tensor_scalar(
                    out=ohT[:], in0=m1T[:], scalar1=iota_sb[:], scalar2=None,
                    op0=ALU.is_equal,
                )

            def sc_m1(nb, scps):
                lo = 512 * nb
                hi = min(512 * (nb + 1), NT - 8)
                mps = scps.tile([C, 512], F32, tag="scps")
                nc.tensor.matmul(
                    out=mps[:, : hi - lo], lhsT=transT_sb[:],
                    rhs=ohT[:, lo + 8 : hi + 8], start=True, stop=True,
                )
                nc.vector.tensor_copy(out=m1T[:, lo:hi], in_=mps[:, : hi - lo])

            def sc_fin():
                nc.vector.tensor_tensor(
                    out=m1T[:, : NT - 8], in0=ohT[:, : NT - 8],
                    in1=m1T[:, : NT - 8], op=ALU.mult,
                )
                nc.vector.reduce_sum(
                    out=scR[:, 0:1], in_=m1T[:, : NT - 8], axis=mybir.AxisListType.X
                )
                nc.vector.tensor_scalar_mul(
                    out=s8T[:, 0, :], in0=ohT[:, 0:GB], scalar1=stv_sb[:]
                )
                nc.vector.reduce_sum(
                    out=scR[:, 1:2], in_=s8T[:, 0, :], axis=mybir.AxisListType.X
                )
                nc.vector.tensor_scalar_mul(
                    out=s8T[:, 1, :], in0=ohT[:, NT - GB : NT], scalar1=env_sb[:]
                )
                nc.vector.reduce_sum(
                    out=scR[:, 2:3], in_=s8T[:, 1, :], axis=mybir.AxisListType.X
                )

            with tc.tile_pool(name="scps", bufs=1, space="PSUM") as scps:
                nc.sync.dma_start(out=labf_sb[:], in_=ins["labf"])
                swork = (
                    [lambda nb=nb: sc_bcast(nb, scps) for nb in range(4)]
                    + [sc_oh]
                    + [lambda nb=nb: sc_m1(nb, scps) for nb in range(4)]
                    + [sc_fin]
                )
                with tc.tile_pool(name="pr1", bufs=1) as pr1:
                    whh1 = load_whh(1, pr1)
                    recurrence(
                        1, G_t, hT1, hTr1, whh1,
                        on_step=make_on_step(1, hTr1, xp_t, work=swork),
                    )
        exchange_last(1, hTr1, xp_t)

        # ---- emissions: em^T [C, NT] = fc @ concat(h2_own, h2_partner) ----
        crf_cm = tc.tile_pool(name="crf", bufs=1)
        crf = crf_cm.__enter__()
        pe_touch_f32(cpack_sb[:, 0:1])
        tileA = crf.tile([C, NT], F32, name="tileA")  # emT
        tileB = crf.tile([C, NT], F32, name="tileB")  # Q
        tileC = crf.tile([C, NT], F32, name="tileC")  # gem
        emT = tileA
        with tc.tile_pool(name="emps", bufs=2, space="PSUM") as emps:
            nc.tensor.ldweights(weights=fcT_sb[:, 0, 0:1])
            emv8 = emT[:].rearrange("c (t b) -> c t b", b=GB)
            for nb in (1, 2, 3, 0):
                for cc in range(2):
                    ps = emps.tile([C, 256], F32, tag=f"emps{cc}")
                    # no DoubleRow: fcT's k-pair stride (C=14B) violates the
                    # dual-fp8 LDWEIGHTS stride%16 ISA restriction
                    ob = slice(256 * nb, 256 * (nb + 1))
                    pb = slice(256 * cc, 256 * (cc + 1))
                    for kk in range(DCH):
                        rhs = (
                            hT1[cc][:, kk, ob]
                            if kk < KCH
                            else xp_t[nb][:, kk - KCH, pb]
                        )
                        nc.tensor.matmul(
                            out=ps[:],
                            lhsT=fcT_sb[:, kk, :],
                            rhs=rhs,
                            start=(kk == 0),
                            stop=(kk == DCH - 1),
                        )
                    nc.vector.tensor_scalar_add(
                        out=emv8[:, 64 * nb : 64 * (nb + 1), 4 * cc : 4 * cc + 4],
                        in0=ps[:].rearrange("c (t b) -> c t b", b=SCH),
                        scalar1=fcb_sb[:],
                    )
        if DEBUG_OUTS:
            nc.sync.dma_start(out=dbg["dbg_em"], in_=emT[:])

        # ---- CRF ----
        with tc.tile_pool(name="crfw", bufs=4) as cw, tc.tile_pool(
            name="crfps", bufs=1, space="PSUM"
        ) as cps:
            Q = tileB
            nc.scalar.activation(out=Q[:], in_=emT[:], func=AF.Exp)
            dve_touch(Q[0:1, 0:1])
            Qv = Q[:].rearrange("c (t b) -> c t b", b=GB)

            # gold emissions total (pd/st/en were precomputed during L1)
            gem = tileC
            nc.vector.tensor_tensor(out=gem[:], in0=emT[:], in1=ohT[:], op=ALU.mult)
            gem_r = cw.tile([C, 1], F32, tag="gred")
            nc.vector.reduce_sum(out=gem_r[:], in_=gem[:], axis=mybir.AxisListType.X)

            score_ps = cps.tile([1, 8], F32, tag="scoreps")
            for i, r in enumerate((gem_r[:], scR[:, 0:1], scR[:, 1:2], scR[:, 2:3])):
                nc.tensor.matmul(
                    out=score_ps[:1, :1],
                    lhsT=onesC1[:],
                    rhs=r,
                    start=(i == 0),
                    stop=(i == 3),
                    skip_group_check=True,
                )
            score_sb = cw.tile([1, 1], F32, tag="scoresb")
            nc.vector.tensor_copy(out=score_sb[:], in_=score_ps[:1, :1])

            # ---- blocked forward chain in exp domain, E pre-scaled by
            # 1/ESCALE so no renorms are needed (f32 range absorbs the
            # drift; the log-correction is added at the end).
            # Block 0 runs the alpha chain over t in [0, BT); blocks k=1..3
            # run transfer-matrix chains Yk = M_k^T = E'D_lo ... E'D_hi
            # (built high-t to low-t), all NBLK chains concurrently.
            expTTp_bf = cw.tile([C, C], BF16, tag="ettbf")
            nc.vector.tensor_copy(out=expTTp_bf[:], in_=expTTp_f32)
            v_prev = cw.tile([C, GB], F32, tag="v")
            nc.vector.tensor_scalar_mul(out=v_prev[:], in0=Qv[:, 0, :], scalar1=expst_sb[:])
            eye_bc = eye_sb.unsqueeze(1).broadcast_to([C, GB, C])
            yps_prev = [None] * NBLK
            for s in range(BT):
                if s > 0:
                    vps = cps.tile([C, GB], F32, tag="vps")
                    nc.tensor.matmul(out=vps[:], lhsT=E_sb[:], rhs=v_prev[:], start=True, stop=True)
                    v_new = cw.tile([C, GB], F32, tag="v")
                    nc.vector.tensor_tensor(out=v_new[:], in0=vps[:], in1=Qv[:, s, :], op=ALU.mult)
                    v_prev = v_new
                for k in range(1, NBLK):
                    t = BT * (k + 1) - 1 - s
                    qb = Qv[:, t, :].unsqueeze(2).broadcast_to([C, GB, C])
                    w = cw.tile([C, GB, C], BF16, tag=f"w{k}")
                    if s == 0:
                        nc.vector.tensor_tensor(out=w[:], in0=eye_bc, in1=qb, op=ALU.mult)
                    else:
                        nc.vector.tensor_tensor(
                            out=w[:],
                            in0=yps_prev[k][:].rearrange("c (b j) -> c b j", b=GB),
                            in1=qb,
                            op=ALU.mult,
                        )
                    yp = cps.tile([C, GB * C], F32, tag=f"yps{k}")
                    nc.tensor.matmul(out=yp[:], lhsT=expTTp_bf[:], rhs=w[:], start=True, stop=True)
                    yps_prev[k] = yp
            yfin = []
            for k in range(1, NBLK):
                yf = cw.tile([C, GB, C], BF16, tag=f"yf{k}")
                nc.vector.tensor_copy(
                    out=yf[:], in_=yps_prev[k][:].rearrange("c (b j) -> c b j", b=GB)
                )
                yfin.append(yf)
            # fold block results into the alpha vector
            for k in range(1, NBLK):
                vb = cw.tile([C, GB], BF16, tag="vb")
                nc.vector.tensor_copy(out=vb[:], in_=v_prev[:])
                aps = cps.tile([C, GB], F32, tag="vps")
                for b in range(GB):
                    nc.tensor.matmul(
                        out=aps[:, b : b + 1],
                        lhsT=yfin[k - 1][:, b, :],
                        rhs=vb[:, b : b + 1],
                        start=(b == 0),
                        stop=(b == GB - 1),
                        skip_group_check=True,
                    )
                v_new = cw.tile([C, GB], F32, tag="v")
                nc.vector.tensor_copy(out=v_new[:], in_=aps[:])
                v_prev = v_new
            vend = cw.tile([C, GB], F32, tag="vend")
            nc.vector.tensor_scalar_mul(out=vend[:], in0=v_prev[:], scalar1=expen_sb[:])
            zps = cps.tile([1, GB], F32, tag="cps1")
            nc.tensor.matmul(out=zps[:], lhsT=onesC1[:], rhs=vend[:], start=True, stop=True)
            lnz = cw.tile([1, GB], F32, tag="lnz")
            nc.scalar.activation(out=lnz[:], in_=zps[:], func=AF.Ln)
            logz = cw.tile([1, GB], F32, tag="logz")
            nc.vector.tensor_scalar(
                out=logz[:], in0=lnz[:], scalar1=float((T - 1) * np.log(ESCALE)),
                scalar2=None, op0=ALU.add,
            )
            lz_tot = cw.tile([1, 1], F32, tag="lztot")
            nc.vector.reduce_sum(out=lz_tot[:], in_=logz[:], axis=mybir.AxisListType.X)
            loss_sb = cw.tile([1, 1], F32, tag="loss_sb")
            nc.vector.tensor_tensor(out=loss_sb[:], in0=lz_tot[:], in1=score_sb[:], op=ALU.subtract)
            nc.sync.dma_start(out=loss_out, in_=loss_sb[:])
            if DEBUG_OUTS:
                dsc = cw.tile([1, 2], F32, tag="dsc")
                nc.vector.tensor_copy(out=dsc[:, 0:1], in_=lz_tot[:])
                nc.vector.tensor_copy(out=dsc[:, 1:2], in_=score_sb[:])
                nc.sync.dma_start(out=dbg["dbg_sc"], in_=dsc[:])
        crf_cm.__exit__(None, None, None)

    est.close()


# ---------------------------------------------------------------------------
# host side
# ---------------------------------------------------------------------------

def make_in_maps(inputs):
    ids = np.asarray(inputs["input_ids"]).astype(np.int64)
    labels = np.asarray(inputs["labels"]).astype(np.int64)
    word_emb = _f32(inputs["word_emb"])
    pos_emb = _f32(inputs["pos_emb"])
    type_emb = _f32(inputs["type_emb"])
    ln_g = _f32(inputs["ln_g"])
    ln_b = _f32(inputs["ln_b"])
    w_ih = _f32(inputs["w_ih"])
    w_hh = _f32(inputs["w_hh"])
    b_ih = _f32(inputs["b_ih"])
    b_hh = _f32(inputs["b_hh"])
    fc_w = _f32(inputs["fc_w"])
    fc_b = _f32(inputs["fc_b"])
    crf_start = _f32(inputs["crf_start"])
    crf_end = _f32(inputs["crf_end"])
    crf_trans = _f32(inputs["crf_trans"])

    posty0 = pos_emb[:T] + type_emb[0][None, :]
    word_emb_bf = _bf(word_emb)

    in_maps = []
    for core in range(NCORES):
        g, d = core // 2, core % 2
        sl = slice(GB * g, GB * (g + 1))
        ids_loc = ids[sl]
        lab_loc = labels[sl]
        posty = posty0
        if d == 1:
            ids_loc = ids_loc[:, ::-1]
            lab_loc = lab_loc[:, ::-1]
            posty = posty0[::-1]

        # layer-0 weights with LN affine folded in (gate-permuted, g x2)
        w0 = _perm_gates(w_ih[0, d] * ln_g[None, :])
        bias0 = _perm_gates((b_ih[0, d] + b_hh[0, d] + w_ih[0, d] @ ln_b)[:, None])[:, 0]
        # layer-1 weights, columns permuted to local [own, partner] order
        w1 = w_ih[1, d]
        if d == 1:
            w1 = np.concatenate([w1[:, HD:], w1[:, :HD]], axis=1)
        w1 = _perm_gates(w1)
        bias1 = _perm_gates((b_ih[1, d] + b_hh[1, d])[:, None])[:, 0]
        fcp = fc_w if d == 0 else np.concatenate([fc_w[:, HD:], fc_w[:, :HD]], axis=1)

        trans_eff = crf_trans if d == 0 else crf_trans.T
        start_eff = crf_start if d == 0 else crf_end
        end_eff = crf_end if d == 0 else crf_start

        pr = 1 - d
        gidx = np.empty((128, KCH), np.int32)
        for cch in range(KCH):
            gidx[:, cch] = pr * (KCH * 128) + cch * 128 + np.arange(128)

        cpack = np.zeros((C, CPW), np.float32)
        cpack[:, 0:C] = np.exp(trans_eff) / ESCALE
        cpack[:, C : 2 * C] = trans_eff.T
        cpack[:, 28] = np.exp(start_eff)
        cpack[:, 29] = np.exp(end_eff)
        cpack[:, 30] = start_eff
        cpack[:, 31] = end_eff
        cpack[:, 32] = np.arange(C, dtype=np.float32)
        cpack[:, 33] = fc_b
        cpack[:, 34 : 34 + C] = (np.exp(trans_eff) / ESCALE).T
        cpack[:, 34 + C : 34 + 2 * C] = np.eye(C, dtype=np.float32)

        b01 = np.concatenate(
            [bias0.reshape(MCH, 128).T, bias1.reshape(MCH, 128).T], axis=1
        )

        # t-major token order: token n = (t, b)
        ids_tm = np.ascontiguousarray(ids_loc.T).reshape(NT, 1)
        lab_tm = np.ascontiguousarray(lab_loc.T).reshape(1, NT)
        posty_rep = np.repeat(np.asarray(posty, np.float32), GB, axis=0)
        in_maps.append(
            dict(
                ids32=np.ascontiguousarray(ids_tm.astype(np.int32)),
                labf=np.ascontiguousarray(lab_tm.astype(np.float32)),
                word_emb=word_emb_bf,
                posty=_bf(posty_rep),
                wih0T=_f8(w0.T),
                wih1T=_f8(w1.T),
                whh0T=_f8(_perm_gates(w_hh[0, d]).T),
                whh1T=_f8(_perm_gates(w_hh[1, d]).T),
                b01=np.ascontiguousarray(b01),
                fcT=_f8(fcp.T),
                cpack=cpack,
                gidx=gidx,
            )
        )
    return in_maps


_PROGRAM = None
_COST_MODEL_NS = None


def _get_program():
    global _PROGRAM, _COST_MODEL_NS
    if _PROGRAM is None:
        _PROGRAM = build_program()
        try:
            from concourse.timeline_sim import TimelineSim

            _COST_MODEL_NS = int(TimelineSim(_PROGRAM, trace=False, no_exec=True).simulate())
        except Exception:
            _COST_MODEL_NS = None
    return _PROGRAM


def run(inputs, trace=False):
    nc = _get_program()
    in_maps = make_in_maps(inputs)
    res = run_bass_kernel_spmd(nc, in_maps, core_ids=list(range(NCORES)), trace=trace)
    total = np.float64(0.0)
    for g in range(4):
        total += np.float64(res.results[2 * g]["loss"][0, 0])
    return np.asarray(total, dtype=np.float32), res


def kernel(**inputs):
    out, _ = run(inputs, trace=False)
    return out

